# revision 9
# baseline (speedup 1.0000x reference)
"""DiffTreeInterpreter scatter-coalesce kernel for 8 Trainium2 cores.

Data-parallel over batch B=32: core c owns batches [4c, 4c+4). All
scatter-adds are device-local. Host work is limited to sharding-style
index prep: bucketing entries by (batch, role-block), and shipping
bit-exact *copies* of per-entry weights (arg_weights / op_dist rows
selected by index) alongside the value stream. All arithmetic
(weight products, value scaling, coalesce sums, stream combine)
happens on the NeuronCores.

Math (see reference): with H = R/2, each entry n (b, l, r, v=mem[n],
w=arg_weights[b,l]) contributes to out[b] at up to 3 bins:
  bin r>>1   with weight op0[b]*w0 if r even, op1[b]*w1 if r odd and r!=1
  bin 2r     with weight op2[b]*w2 (only r < H)
  bin 2r+1   with weight op2[b]*w3 (only r < H)
plus out[b,1] += op2[b]*root_filler[b].
(The reference's pad-mask is a no-op on values: masked rows are all-zero.)

Device algorithm per core: entries are bucketed into 128-entry tiles
aligned to role windows. For each tile, build u-scaled one-hot
matrices O[p, j] = u[p] * (iota[j] == r_rel[p]) on the Vector engine
(one fused tensor_scalar each), and matmul O^T @ V on the PE into a
PSUM block of 128 output bins. PSUM blocks drain to SBUF staging and
DMA out contiguously.
"""

import sys

if "/opt/trn_rl_repo" not in sys.path:
    sys.path.insert(0, "/opt/trn_rl_repo")

import numpy as np

B, L, F, R = 32, 128, 128, 4096
H = R >> 1
N = 262144
NCORES = 8
BPC = B // NCORES  # batches per core

P = 128  # partitions / tile entry count / bin-block size

# Static schedule: per batch,
#  lower half (r < 2048): 32 role-blocks of 64 r-values -> bins [128k, 128k+128)
#     via cons streams (2r, 2r+1); capacity LOW_CAP tiles each.
#  upper half (r >= 2048): 8 role-blocks of 256 r-values, car/cdr stream only;
#     capacity UP_CAP tiles each.
#  car/cdr stream of lower-half r also feeds S1 blocks (bins r>>1).
LOW_CAP = 2  # tiles per (batch, 64-r block), holds <= 256 entries (data max 161+1)
UP_CAP = 5  # tiles per (batch, 256-r block), holds <= 640 entries (data max 575)
TILES_PER_BATCH = 32 * LOW_CAP + 8 * UP_CAP  # 104
NT = BPC * TILES_PER_BATCH  # tiles per core (416)

# meta channels
MC_R1, MC_WA, MC_OPA, MC_R23, MC_WB, MC_WC, MC_OP2, MC_PAD = range(8)
NMC = 8

_PROG_CACHE = {}

# Tunables for A/B experiments. val_dtype fp16 trades ~5e-4 relative error
# for 1-pass PE matmuls and half the value-stream DMA traffic.
CONFIG = {
    "val_dtype": "float16",   # dtype of values + one-hots through the PE
    "o1_engine": "gpsimd",    # engine for car/cdr one-hot builds
    "o23_engine": "vector",   # engine for cons one-hot builds
    "copy_engine": "vector",  # PSUM -> SBUF stage copies
    "out_dma": "scalar",      # queue for output writes
    "vload_batch": 4,         # value tiles per load DMA
}


def _tile_base(b, lower, blk):
    """Global tile index of slot 0 of a block within a core's stream."""
    base = b * TILES_PER_BATCH
    if lower:
        return base + blk * LOW_CAP
    return base + 32 * LOW_CAP + blk * UP_CAP


def _build_program():
    import concourse.bacc as bacc
    import concourse.mybir as mybir
    import concourse.tile as tile

    fp32 = mybir.dt.float32
    vdt = getattr(mybir.dt, CONFIG["val_dtype"])
    EQ = mybir.AluOpType.is_equal
    MUL = mybir.AluOpType.mult
    ADD = mybir.AluOpType.add
    VB = CONFIG["vload_batch"]
    assert TILES_PER_BATCH % VB == 0

    nc = bacc.Bacc(None, target_bir_lowering=False)
    vals = nc.dram_tensor("vals", [NT * P, F], vdt, kind="ExternalInput")
    meta = nc.dram_tensor("meta", [NT, P, NMC], fp32, kind="ExternalInput")
    iota = nc.dram_tensor("iota", [P, P], vdt, kind="ExternalInput")
    out = nc.dram_tensor("out", [BPC, R, F], fp32, kind="ExternalOutput")

    vals_t = vals.rearrange("(t p) f -> t p f", p=P)

    with tile.TileContext(nc) as tc:
        with tc.tile_pool(name="const", bufs=1) as cpool, \
             tc.tile_pool(name="metap", bufs=2) as mpool, \
             tc.tile_pool(name="useq", bufs=2) as upool, \
             tc.tile_pool(name="vload", bufs=4) as vpool, \
             tc.tile_pool(name="ohot", bufs=8) as opool, \
             tc.tile_pool(name="stage", bufs=14) as spool, \
             tc.tile_pool(name="ps1", bufs=2, space="PSUM") as ps1pool, \
             tc.tile_pool(name="ps23", bufs=2, space="PSUM") as ps23pool:

            o1_eng = getattr(nc, CONFIG["o1_engine"])
            o23_eng = getattr(nc, CONFIG["o23_engine"])
            copy_eng = getattr(nc, CONFIG["copy_engine"])
            out_eng = getattr(nc, CONFIG["out_dma"])

            io_t = cpool.tile([P, P], vdt)
            nc.sync.dma_start(out=io_t[:], in_=iota[:])

            vtiles = {}  # batched value loads: vload index -> sbuf tile

            for b in range(BPC):
                # whole batch's metadata: [104, 128, 8] -> SBUF [128, 104, 8]
                m = mpool.tile([P, TILES_PER_BATCH, NMC], fp32)
                nc.sync.dma_start(
                    out=m[:],
                    in_=meta[b * TILES_PER_BATCH:(b + 1) * TILES_PER_BATCH]
                    .rearrange("t p c -> p t c"),
                )
                # u slabs [128, 104] in val dtype: u1 = wA*opA, etc.
                u1 = upool.tile([P, TILES_PER_BATCH], fp32, tag="u1")
                u2 = upool.tile([P, TILES_PER_BATCH], fp32, tag="u2")
                u3 = upool.tile([P, TILES_PER_BATCH], fp32, tag="u3")
                nc.vector.tensor_tensor(
                    out=u1[:], in0=m[:, :, MC_WA], in1=m[:, :, MC_OPA], op=MUL)
                nc.vector.tensor_tensor(
                    out=u2[:], in0=m[:, :, MC_WB], in1=m[:, :, MC_OP2], op=MUL)
                nc.vector.tensor_tensor(
                    out=u3[:], in0=m[:, :, MC_WC], in1=m[:, :, MC_OP2], op=MUL)
                r1s = m[:, :, MC_R1]
                r23s = m[:, :, MC_R23]

                stage = {}

                def vtile(t):
                    """SBUF view of value tile t (batched loads of VB tiles)."""
                    tg = b * TILES_PER_BATCH + t
                    g = tg // VB
                    if g not in vtiles:
                        vt = vpool.tile([P, VB, F], vdt, tag="v")
                        nc.sync.dma_start(
                            out=vt[:],
                            in_=vals_t[g * VB:(g + 1) * VB]
                            .rearrange("q p f -> p q f"))
                        vtiles[g] = vt
                    return vtiles[g][:, tg % VB, :]

                def do_tile(t, ps1, s1_start, s1_stop, ps23=None,
                            s23_start=False, s23_stop=False):
                    v = vtile(t)
                    # car/cdr one-hot: O1[p, j] = u1[p] * (iota[p,j] == r1_rel[p])
                    o1 = opool.tile([P, P], vdt, tag="o1")
                    o1_eng.tensor_scalar(
                        out=o1[:], in0=io_t[:],
                        scalar1=r1s[:, t:t + 1], scalar2=u1[:, t:t + 1],
                        op0=EQ, op1=MUL)
                    nc.tensor.matmul(out=ps1[:], lhsT=o1[:], rhs=v,
                                     start=s1_start, stop=s1_stop)
                    if ps23 is not None:
                        # cons one-hots, interleaved into even/odd columns
                        o23 = opool.tile([P, 64, 2], vdt, tag="o23")
                        o23_eng.tensor_scalar(
                            out=o23[:, :, 0], in0=io_t[:, 0:64],
                            scalar1=r23s[:, t:t + 1], scalar2=u2[:, t:t + 1],
                            op0=EQ, op1=MUL)
                        o23_eng.tensor_scalar(
                            out=o23[:, :, 1], in0=io_t[:, 0:64],
                            scalar1=r23s[:, t:t + 1], scalar2=u3[:, t:t + 1],
                            op0=EQ, op1=MUL)
                        nc.tensor.matmul(
                            out=ps23[:],
                            lhsT=o23[:].rearrange("p a t -> p (a t)"),
                            rhs=v, start=s23_start, stop=s23_stop)

                for bk1 in range(16):
                    ps1 = ps1pool.tile([P, F], fp32, tag="ps1")
                    if bk1 < 8:
                        nmm = 4 * LOW_CAP
                        i1 = 0
                        for j in range(4):
                            k = 4 * bk1 + j
                            ps23 = ps23pool.tile([P, F], fp32, tag="ps23")
                            for s in range(LOW_CAP):
                                t = _tile_base(0, True, k) + s
                                do_tile(t, ps1, i1 == 0, i1 == nmm - 1,
                                        ps23, s == 0, s == LOW_CAP - 1)
                                i1 += 1
                            st = spool.tile([P, F], fp32, tag="st")
                            if CONFIG["copy_engine"] == "scalar":
                                copy_eng.copy(out=st[:], in_=ps23[:])
                            else:
                                copy_eng.tensor_copy(out=st[:], in_=ps23[:])
                            stage[k] = st
                            if k >= 16:
                                out_eng.dma_start(
                                    out=out[b, k * P:(k + 1) * P, :], in_=st[:])
                    else:
                        ub = bk1 - 8
                        for s in range(UP_CAP):
                            t = _tile_base(0, False, ub) + s
                            do_tile(t, ps1, s == 0, s == UP_CAP - 1)
                    # car/cdr drain: bins [128*bk1, +128) -> add into stage
                    st = stage[bk1]
                    nc.vector.tensor_tensor(
                        out=st[:], in0=st[:], in1=ps1[:], op=ADD)
                    out_eng.dma_start(
                        out=out[b, bk1 * P:(bk1 + 1) * P, :], in_=st[:])

    nc.compile()
    return nc


def _pack_inputs(mem_values, arg_weights, root_filler, op_dist,
                 batch_idx, slot_idx, role_idx):
    """Host-side sharding/packing. Index selection and copies only."""
    mem_values = np.ascontiguousarray(mem_values, dtype=np.float32)
    arg_weights = np.asarray(arg_weights, dtype=np.float32)
    root_filler = np.asarray(root_filler, dtype=np.float32)
    op_dist = np.asarray(op_dist, dtype=np.float32)
    batch_idx = np.asarray(batch_idx, dtype=np.int64)
    slot_idx = np.asarray(slot_idx, dtype=np.int64)
    role_idx = np.asarray(role_idx, dtype=np.int64)

    # per-entry selected copies (pure gathers, no arithmetic)
    w = arg_weights[batch_idx, slot_idx]  # [N, 4] copies
    r = role_idx
    even = (r & 1) == 0
    wA = np.where(even, w[:, 0], np.where(r != 1, w[:, 1], 0.0)).astype(np.float32)
    opA = np.where(even, op_dist[batch_idx, 0], op_dist[batch_idx, 1]).astype(np.float32)
    lower = r < H
    wB = np.where(lower, w[:, 2], 0.0).astype(np.float32)
    wC = np.where(lower, w[:, 3], 0.0).astype(np.float32)
    op2c = op_dist[batch_idx, 2].astype(np.float32)

    # block id within batch-stream: lower blocks 0..31 (64 r each),
    # upper blocks 32..39 (256 r each)
    blk = np.where(lower, r >> 6, 32 + ((r - H) >> 8))
    # capacity slots per block
    cap_slots = np.concatenate([
        np.full(32, LOW_CAP * P, np.int64), np.full(8, UP_CAP * P, np.int64)])
    blk_slot0 = np.concatenate([[0], np.cumsum(cap_slots)])[:-1]  # [40]

    vdt = np.dtype(CONFIG["val_dtype"])
    in_maps = []
    for c in range(NCORES):
        vals_s = np.zeros((NT * P, F), vdt)
        meta_s = np.zeros((NT, P, NMC), np.float32)
        meta_s[:, :, MC_R1] = -1.0
        meta_s[:, :, MC_R23] = -1.0
        for bb in range(BPC):
            b = c * BPC + bb
            sel = np.nonzero(batch_idx == b)[0]
            gb = blk[sel]
            order = np.argsort(gb, kind="stable")
            sel = sel[order]
            gb = gb[order]
            counts = np.bincount(gb, minlength=40)
            if (counts[:32] > LOW_CAP * P - 1).any() or (counts[32:] > UP_CAP * P).any():
                raise RuntimeError(
                    "static schedule capacity exceeded; "
                    f"counts max lower={counts[:32].max()} upper={counts[32:].max()}")
            first = np.concatenate([[0], np.cumsum(counts)])[:-1]
            pos_in_blk = np.arange(sel.size) - first[gb]
            slot = blk_slot0[gb] + pos_in_blk  # slot within the batch stream
            slot += bb * TILES_PER_BATCH * P
            vals_s[slot] = mem_values[sel]
            tix, pix = slot // P, slot % P
            rr = role_idx[sel]
            meta_s[tix, pix, MC_R1] = ((rr >> 1) & 127).astype(np.float32)
            meta_s[tix, pix, MC_WA] = wA[sel]
            meta_s[tix, pix, MC_OPA] = opA[sel]
            meta_s[tix, pix, MC_R23] = np.where(rr < H, (rr & 63), -1).astype(np.float32)
            meta_s[tix, pix, MC_WB] = wB[sel]
            meta_s[tix, pix, MC_WC] = wC[sel]
            meta_s[tix, pix, MC_OP2] = op2c[sel]
            # synthetic root entry -> bin 1 == 2*0+1 (block 0, odd cons slot)
            rslot = bb * TILES_PER_BATCH * P + counts[0]
            vals_s[rslot] = root_filler[b]
            ti, pi = rslot // P, rslot % P
            meta_s[ti, pi, MC_R1] = -1.0
            meta_s[ti, pi, MC_R23] = 0.0
            meta_s[ti, pi, MC_WC] = 1.0
            meta_s[ti, pi, MC_OP2] = op_dist[b, 2]
        in_maps.append({
            "vals": vals_s,
            "meta": meta_s,
            "iota": np.broadcast_to(np.arange(P, dtype=vdt), (P, P)).copy(),
        })
    return in_maps


def kernel(**inputs):
    from concourse.bass_utils import run_bass_kernel_spmd

    in_maps = _pack_inputs(**inputs)
    if "nc" not in _PROG_CACHE:
        _PROG_CACHE["nc"] = _build_program()
    nc = _PROG_CACHE["nc"]
    res = run_bass_kernel_spmd(nc, in_maps, list(range(NCORES)))
    return np.concatenate([res.results[c]["out"] for c in range(NCORES)], axis=0)


# revision 10
# speedup vs baseline: 5.1754x; 5.1754x over previous
"""DiffTreeInterpreter scatter-coalesce kernel for 8 Trainium2 cores.

Data-parallel over batch B=32: core c owns batches [4c, 4c+4). All
scatter-adds are device-local. Host work is limited to sharding-style
index prep: bucketing entries by (batch, role-block), and shipping
bit-exact *copies* of per-entry weights (arg_weights / op_dist rows
selected by index) alongside the value stream. All arithmetic
(weight products, value scaling, coalesce sums, stream combine)
happens on the NeuronCores.

Math (see reference): with H = R/2, each entry n (b, l, r, v=mem[n],
w=arg_weights[b,l]) contributes to out[b] at up to 3 bins:
  bin r>>1   with weight op0[b]*w0 if r even, op1[b]*w1 if r odd and r!=1
  bin 2r     with weight op2[b]*w2 (only r < H)
  bin 2r+1   with weight op2[b]*w3 (only r < H)
plus out[b,1] += op2[b]*root_filler[b].
(The reference's pad-mask is a no-op on values: masked rows are all-zero.)

Device algorithm per core: entries are bucketed into 128-entry tiles
aligned to role windows; tiles are organized into 16 groups per batch
(8 "lower" groups of 8 tiles covering r<2048, feeding both the
car/cdr stream and the interleaved cons stream; 8 "upper" groups of
5 tiles covering r>=2048, car/cdr only). Per group, GPSIMD
local_scatter builds u-scaled one-hot slabs in fp16 (u = weight
products computed on the Vector engine); the PE contracts one-hot^T @
values into PSUM blocks of 128 output bins; PSUM drains into a
per-batch SBUF output region (ACT copies + DVE adds) which is written
out with one DMA per batch.
"""

import sys

if "/opt/trn_rl_repo" not in sys.path:
    sys.path.insert(0, "/opt/trn_rl_repo")

import numpy as np

B, L, F, R = 32, 128, 128, 4096
H = R >> 1
N = 262144
NCORES = 8
BPC = B // NCORES  # batches per core

P = 128  # partitions / tile entry count / bin-block size

# Static schedule per batch: 16 groups; lower groups g<8 have 8 tiles
# (4 cons blocks x 2 tiles, r in [256g, 256g+256)); upper groups 5 tiles.
NG = 16
LOW_CAP = 2   # tiles per (batch, 64-r block); holds <= 256 entries
UP_CAP = 5    # tiles per (batch, 256-r block); holds <= 640 entries
TILES_PER_BATCH = 32 * LOW_CAP + 8 * UP_CAP  # 104
NSLOT = NG * 8  # group-padded slot space (upper groups use 5 of 8)
NT = BPC * TILES_PER_BATCH  # tiles per core (416)

# meta channels (fp32, slot space)
MC_WA, MC_OPA, MC_WB, MC_WC, MC_OP2, MC_R1, MC_R23, MC_PAD = range(8)
NMC = 8

_PROG_CACHE = {}

CONFIG = {
    "val_dtype": "float16",  # PE operand dtype (values + one-hots)
    "vload_batch": 8,        # value tiles per load DMA
}


def _slot_of(g, tloc):
    return g * 8 + tloc


def _tile_of(g, tloc):
    if g < 8:
        return g * 8 + tloc
    return 64 + (g - 8) * UP_CAP + tloc


def _build_program():
    import concourse.bacc as bacc
    import concourse.mybir as mybir
    import concourse.tile as tile

    fp32 = mybir.dt.float32
    i16 = mybir.dt.int16
    vdt = getattr(mybir.dt, CONFIG["val_dtype"])
    MUL = mybir.AluOpType.mult
    ADD = mybir.AluOpType.add
    VB = CONFIG["vload_batch"]
    assert TILES_PER_BATCH % VB == 0

    nc = bacc.Bacc(None, target_bir_lowering=False)
    vals = nc.dram_tensor("vals", [NT * P, F], vdt, kind="ExternalInput")
    meta = nc.dram_tensor("meta", [BPC, NSLOT, P, NMC], fp32,
                          kind="ExternalInput")
    idx1 = nc.dram_tensor("idx1", [BPC, NG, P, 8], i16, kind="ExternalInput")
    idx23 = nc.dram_tensor("idx23", [BPC, 8, P, 16], i16, kind="ExternalInput")
    out = nc.dram_tensor("out", [BPC, R, F], fp32, kind="ExternalOutput")

    vals_t = vals.rearrange("(t p) f -> t p f", p=P)

    with tile.TileContext(nc) as tc:
        with tc.tile_pool(name="metap", bufs=2) as mpool, \
             tc.tile_pool(name="useq", bufs=2) as upool, \
             tc.tile_pool(name="u23p", bufs=4) as u23pool, \
             tc.tile_pool(name="vload", bufs=4) as vpool, \
             tc.tile_pool(name="ohot", bufs=3) as opool, \
             tc.tile_pool(name="outreg", bufs=2) as rpool, \
             tc.tile_pool(name="ps1", bufs=2, space="PSUM") as ps1pool, \
             tc.tile_pool(name="ps23", bufs=2, space="PSUM") as ps23pool:

            vtiles = {}

            for b in range(BPC):
                m = mpool.tile([P, NSLOT, NMC], fp32)
                nc.sync.dma_start(
                    out=m[:], in_=meta[b].rearrange("s p c -> p s c"))
                x1 = mpool.tile([P, NG, 8], i16, tag="x1")
                nc.sync.dma_start(
                    out=x1[:], in_=idx1[b].rearrange("g p s -> p g s"))
                x23 = mpool.tile([P, 8, 16], i16, tag="x23")
                nc.sync.dma_start(
                    out=x23[:], in_=idx23[b].rearrange("g p s -> p g s"))

                # u1 slab over all slots (fp16, data for O1 scatters)
                u1 = upool.tile([P, NSLOT], vdt, tag="u1")
                nc.vector.tensor_tensor(
                    out=u1[:], in0=m[:, :, MC_WA], in1=m[:, :, MC_OPA], op=MUL)

                outreg = rpool.tile([P, 32 * P], fp32)

                def vtile(t):
                    tg = b * TILES_PER_BATCH + t
                    g = tg // VB
                    if g not in vtiles:
                        vt = vpool.tile([P, VB, F], vdt, tag="v")
                        nc.sync.dma_start(
                            out=vt[:],
                            in_=vals_t[g * VB:(g + 1) * VB]
                            .rearrange("q p f -> p q f"))
                        vtiles[g] = vt
                    return vtiles[g][:, tg % VB, :]

                for g in range(NG):
                    lower = g < 8
                    ntiles = 8 if lower else UP_CAP
                    ps1 = ps1pool.tile([P, F], fp32, tag="ps1")
                    # group one-hot slabs via GPSIMD local scatter
                    o1s = opool.tile([P, 8 * P], vdt, tag="o1s")
                    nc.gpsimd.local_scatter(
                        out_ap=o1s[:, :ntiles * P],
                        data_ap=u1[:, g * 8:g * 8 + 8],
                        idxs_ap=x1[:, g, :],
                        channels=P, num_elems=ntiles * P, num_idxs=8)
                    if lower:
                        u23g = u23pool.tile([P, 16], vdt, tag="u23g")
                        nc.vector.tensor_tensor(
                            out=u23g[:, 0:8],
                            in0=m[:, g * 8:g * 8 + 8, MC_WB],
                            in1=m[:, g * 8:g * 8 + 8, MC_OP2], op=MUL)
                        nc.vector.tensor_tensor(
                            out=u23g[:, 8:16],
                            in0=m[:, g * 8:g * 8 + 8, MC_WC],
                            in1=m[:, g * 8:g * 8 + 8, MC_OP2], op=MUL)
                        o23s = opool.tile([P, 8 * P], vdt, tag="o23s")
                        nc.gpsimd.local_scatter(
                            out_ap=o23s[:], data_ap=u23g[:],
                            idxs_ap=x23[:, g, :],
                            channels=P, num_elems=8 * P, num_idxs=16)
                    ps23 = None
                    for tloc in range(ntiles):
                        v = vtile(_tile_of(g, tloc))
                        nc.tensor.matmul(
                            out=ps1[:], lhsT=o1s[:, tloc * P:(tloc + 1) * P],
                            rhs=v, start=(tloc == 0), stop=(tloc == ntiles - 1))
                        if lower:
                            if tloc % 2 == 0:
                                ps23 = ps23pool.tile([P, F], fp32, tag="ps23")
                            nc.tensor.matmul(
                                out=ps23[:],
                                lhsT=o23s[:, tloc * P:(tloc + 1) * P],
                                rhs=v, start=(tloc % 2 == 0),
                                stop=(tloc % 2 == 1))
                            if tloc % 2 == 1:
                                k = 4 * g + tloc // 2
                                nc.scalar.copy(
                                    out=outreg[:, k * P:(k + 1) * P],
                                    in_=ps23[:])
                    # car/cdr drain: bins [128g, +128) add onto cons copy
                    nc.vector.tensor_tensor(
                        out=outreg[:, g * P:(g + 1) * P],
                        in0=outreg[:, g * P:(g + 1) * P], in1=ps1[:], op=ADD)
                nc.sync.dma_start(
                    out=out[b].rearrange("(k p) f -> p k f", p=P),
                    in_=outreg[:].rearrange("p (k f) -> p k f", f=F))

    nc.compile()
    return nc


def _pack_inputs(mem_values, arg_weights, root_filler, op_dist,
                 batch_idx, slot_idx, role_idx):
    """Host-side sharding/packing. Index selection and copies only."""
    mem_values = np.ascontiguousarray(mem_values, dtype=np.float32)
    arg_weights = np.asarray(arg_weights, dtype=np.float32)
    root_filler = np.asarray(root_filler, dtype=np.float32)
    op_dist = np.asarray(op_dist, dtype=np.float32)
    batch_idx = np.asarray(batch_idx, dtype=np.int64)
    slot_idx = np.asarray(slot_idx, dtype=np.int64)
    role_idx = np.asarray(role_idx, dtype=np.int64)

    # per-entry selected copies (pure gathers, no arithmetic)
    w = arg_weights[batch_idx, slot_idx]  # [N, 4] copies
    r = role_idx
    even = (r & 1) == 0
    wA = np.where(even, w[:, 0], np.where(r != 1, w[:, 1], 0.0)).astype(np.float32)
    opA = np.where(even, op_dist[batch_idx, 0],
                   op_dist[batch_idx, 1]).astype(np.float32)
    lo = r < H
    wB = np.where(lo, w[:, 2], 0.0).astype(np.float32)
    wC = np.where(lo, w[:, 3], 0.0).astype(np.float32)
    op2c = op_dist[batch_idx, 2].astype(np.float32)

    # block id within batch: lower cons blocks 0..31 (64 r each),
    # upper blocks 32..39 (256 r each)
    blk = np.where(lo, r >> 6, 32 + ((r - H) >> 8))
    cap_slots = np.concatenate([
        np.full(32, LOW_CAP * P, np.int64), np.full(8, UP_CAP * P, np.int64)])
    blk_slot0 = np.concatenate([[0], np.cumsum(cap_slots)])[:-1]  # [40]

    vdt = np.dtype(CONFIG["val_dtype"])
    in_maps = []
    for c in range(NCORES):
        vals_s = np.zeros((NT * P, F), vdt)
        # entry-indexed (tile space) scratch, converted to slot space below
        r1_rel = np.full((NT, P), -1, np.int64)
        r23_rel = np.full((NT, P), -1, np.int64)
        wA_t = np.zeros((NT, P), np.float32)
        opA_t = np.zeros((NT, P), np.float32)
        wB_t = np.zeros((NT, P), np.float32)
        wC_t = np.zeros((NT, P), np.float32)
        op2_t = np.zeros((NT, P), np.float32)
        for bb in range(BPC):
            b = c * BPC + bb
            sel = np.nonzero(batch_idx == b)[0]
            gb = blk[sel]
            order = np.argsort(gb, kind="stable")
            sel = sel[order]
            gb = gb[order]
            counts = np.bincount(gb, minlength=40)
            if (counts[:32] > LOW_CAP * P - 1).any() or \
               (counts[32:] > UP_CAP * P).any():
                raise RuntimeError(
                    "static schedule capacity exceeded: "
                    f"lower={counts[:32].max()} upper={counts[32:].max()}")
            first = np.concatenate([[0], np.cumsum(counts)])[:-1]
            pos = np.arange(sel.size) - first[gb]
            slot = blk_slot0[gb] + pos + bb * TILES_PER_BATCH * P
            vals_s[slot] = mem_values[sel]
            tix, pix = slot // P, slot % P
            rr = role_idx[sel]
            r1_rel[tix, pix] = (rr >> 1) & 127
            r23_rel[tix, pix] = np.where(rr < H, rr & 63, -1)
            wA_t[tix, pix] = wA[sel]
            opA_t[tix, pix] = opA[sel]
            wB_t[tix, pix] = wB[sel]
            wC_t[tix, pix] = wC[sel]
            op2_t[tix, pix] = op2c[sel]
            # synthetic root entry -> bin 1 == 2*0+1 (block 0, odd cons)
            rslot = bb * TILES_PER_BATCH * P + counts[0]
            vals_s[rslot] = root_filler[b]
            ti, pi = rslot // P, rslot % P
            r1_rel[ti, pi] = -1
            r23_rel[ti, pi] = 0
            wC_t[ti, pi] = 1.0
            op2_t[ti, pi] = op_dist[b, 2]

        # tile space -> slot space
        meta_s = np.zeros((BPC, NSLOT, P, NMC), np.float32)
        idx1_s = np.full((BPC, NG, P, 8), -1, np.int16)
        idx23_s = np.full((BPC, 8, P, 16), -1, np.int16)
        for bb in range(BPC):
            for g in range(NG):
                ntl = 8 if g < 8 else UP_CAP
                for tloc in range(ntl):
                    t = bb * TILES_PER_BATCH + _tile_of(g, tloc)
                    s = _slot_of(g, tloc)
                    meta_s[bb, s, :, MC_WA] = wA_t[t]
                    meta_s[bb, s, :, MC_OPA] = opA_t[t]
                    meta_s[bb, s, :, MC_WB] = wB_t[t]
                    meta_s[bb, s, :, MC_WC] = wC_t[t]
                    meta_s[bb, s, :, MC_OP2] = op2_t[t]
                    meta_s[bb, s, :, MC_R1] = r1_rel[t]
                    meta_s[bb, s, :, MC_R23] = r23_rel[t]
                    v1 = r1_rel[t] >= 0
                    idx1_s[bb, g, :, tloc] = np.where(
                        v1, tloc * P + r1_rel[t], -1)
                    if g < 8:
                        v23 = r23_rel[t] >= 0
                        base = tloc * P + 2 * r23_rel[t]
                        idx23_s[bb, g, :, tloc] = np.where(v23, base, -1)
                        idx23_s[bb, g, :, 8 + tloc] = np.where(v23, base + 1, -1)

        in_maps.append({
            "vals": vals_s,
            "meta": meta_s,
            "idx1": idx1_s,
            "idx23": idx23_s,
        })
    return in_maps


def kernel(**inputs):
    from concourse.bass_utils import run_bass_kernel_spmd

    in_maps = _pack_inputs(**inputs)
    if "nc" not in _PROG_CACHE:
        _PROG_CACHE["nc"] = _build_program()
    nc = _PROG_CACHE["nc"]
    res = run_bass_kernel_spmd(nc, in_maps, list(range(NCORES)))
    return np.concatenate([res.results[c]["out"] for c in range(NCORES)], axis=0)


# revision 11
# speedup vs baseline: 5.6595x; 1.0935x over previous
"""DiffTreeInterpreter scatter-coalesce kernel for 8 Trainium2 cores.

Data-parallel over batch B=32: core c owns batches [4c, 4c+4). All
scatter-adds are device-local. Host work is limited to sharding-style
index prep: bucketing entries by (batch, role-block), and shipping
bit-exact *copies* of per-entry weights (arg_weights / op_dist rows
selected by index) alongside the value stream. All arithmetic
(weight products, value scaling, coalesce sums, stream combine)
happens on the NeuronCores.

Math (see reference): with H = R/2, each entry n (b, l, r, v=mem[n],
w=arg_weights[b,l]) contributes to out[b] at up to 3 bins:
  bin r>>1   with weight op0[b]*w0 if r even, op1[b]*w1 if r odd and r!=1
  bin 2r     with weight op2[b]*w2 (only r < H)
  bin 2r+1   with weight op2[b]*w3 (only r < H)
plus out[b,1] += op2[b]*root_filler[b].
(The reference's pad-mask is a no-op on values: masked rows are all-zero.)

Device algorithm per core: entries are bucketed into 128-entry tiles
aligned to role windows; tiles are organized into 16 groups per batch
(8 "lower" groups of 8 tiles covering r<2048, feeding both the
car/cdr stream and the interleaved cons stream; 8 "upper" groups of
5 tiles covering r>=2048, car/cdr only). Per group, GPSIMD
local_scatter builds u-scaled one-hot slabs in fp16 (u = weight
products computed on the Vector engine); the PE contracts one-hot^T @
values into PSUM blocks of 128 output bins; PSUM drains into a
per-batch SBUF output region (ACT copies + DVE adds) which is written
out with one DMA per batch.
"""

import sys

if "/opt/trn_rl_repo" not in sys.path:
    sys.path.insert(0, "/opt/trn_rl_repo")

import numpy as np

B, L, F, R = 32, 128, 128, 4096
H = R >> 1
N = 262144
NCORES = 8
BPC = B // NCORES  # batches per core

P = 128  # partitions / tile entry count / bin-block size

# Static schedule per batch: 16 groups; lower groups g<8 have 8 tiles
# (4 cons blocks x 2 tiles, r in [256g, 256g+256)); upper groups 5 tiles.
NG = 16
LOW_CAP = 2   # tiles per (batch, 64-r block); holds <= 256 entries
UP_CAP = 5    # tiles per (batch, 256-r block); holds <= 640 entries
TILES_PER_BATCH = 32 * LOW_CAP + 8 * UP_CAP  # 104
NSLOT = NG * 8  # group-padded slot space (upper groups use 5 of 8)
NT = BPC * TILES_PER_BATCH  # tiles per core (416)

# meta channels (fp32, slot space)
MC_WA, MC_OPA, MC_WB, MC_WC, MC_OP2, MC_R1, MC_R23, MC_PAD = range(8)
NMC = 8

_PROG_CACHE = {}

CONFIG = {
    "val_dtype": "float16",  # PE operand dtype (values + one-hots)
    "vload_batch": 8,        # value tiles per load DMA
}


def _slot_of(g, tloc):
    return g * 8 + tloc


def _tile_of(g, tloc):
    if g < 8:
        return g * 8 + tloc
    return 64 + (g - 8) * UP_CAP + tloc


def _build_program():
    import concourse.bacc as bacc
    import concourse.mybir as mybir
    import concourse.tile as tile

    fp32 = mybir.dt.float32
    i16 = mybir.dt.int16
    vdt = getattr(mybir.dt, CONFIG["val_dtype"])
    MUL = mybir.AluOpType.mult
    ADD = mybir.AluOpType.add
    VB = CONFIG["vload_batch"]
    assert TILES_PER_BATCH % VB == 0

    nc = bacc.Bacc(None, target_bir_lowering=False)
    vals = nc.dram_tensor("vals", [NT * P, F], vdt, kind="ExternalInput")
    meta = nc.dram_tensor("meta", [BPC, NSLOT, P, NMC], fp32,
                          kind="ExternalInput")
    idx1 = nc.dram_tensor("idx1", [BPC, NG, P, 8], i16, kind="ExternalInput")
    idx23 = nc.dram_tensor("idx23", [BPC, 8, P, 16], i16, kind="ExternalInput")
    out = nc.dram_tensor("out", [BPC, R, F], fp32, kind="ExternalOutput")

    vals_t = vals.rearrange("(t p) f -> t p f", p=P)

    with tile.TileContext(nc) as tc:
        with tc.tile_pool(name="metap", bufs=BPC) as mpool, \
             tc.tile_pool(name="useq", bufs=BPC) as upool, \
             tc.tile_pool(name="u23p", bufs=8) as u23pool, \
             tc.tile_pool(name="vload", bufs=8) as vpool, \
             tc.tile_pool(name="ohot", bufs=8) as opool, \
             tc.tile_pool(name="outreg", bufs=2) as rpool, \
             tc.tile_pool(name="ps1", bufs=3, space="PSUM") as ps1pool, \
             tc.tile_pool(name="ps23", bufs=4, space="PSUM") as ps23pool:

            vtiles = {}

            # prefetch all batches' metadata up front (small, keeps the
            # batch-transition critical path off the DMA queue)
            metas = []
            for b in range(BPC):
                m = mpool.tile([P, NSLOT, NMC], fp32, tag="m")
                nc.sync.dma_start(
                    out=m[:], in_=meta[b].rearrange("s p c -> p s c"))
                x1 = mpool.tile([P, NG, 8], i16, tag="x1")
                nc.sync.dma_start(
                    out=x1[:], in_=idx1[b].rearrange("g p s -> p g s"))
                x23 = mpool.tile([P, 8, 16], i16, tag="x23")
                nc.sync.dma_start(
                    out=x23[:], in_=idx23[b].rearrange("g p s -> p g s"))
                u1 = upool.tile([P, NSLOT], vdt, tag="u1")
                nc.vector.tensor_tensor(
                    out=u1[:], in0=m[:, :, MC_WA], in1=m[:, :, MC_OPA], op=MUL)
                metas.append((m, x1, x23, u1))

            for b in range(BPC):
                m, x1, x23, u1 = metas[b]
                outreg = rpool.tile([P, 32 * P], fp32)

                def vtile(t):
                    tg = b * TILES_PER_BATCH + t
                    g = tg // VB
                    if g not in vtiles:
                        vt = vpool.tile([P, VB, F], vdt, tag="v")
                        nc.sync.dma_start(
                            out=vt[:],
                            in_=vals_t[g * VB:(g + 1) * VB]
                            .rearrange("q p f -> p q f"))
                        vtiles[g] = vt
                    return vtiles[g][:, tg % VB, :]

                for g in range(NG):
                    lower = g < 8
                    ntiles = 8 if lower else UP_CAP
                    ps1 = ps1pool.tile([P, F], fp32, tag="ps1")
                    # group one-hot slabs via GPSIMD local scatter
                    o1s = opool.tile([P, 8 * P], vdt, tag="o1s")
                    nc.gpsimd.local_scatter(
                        out_ap=o1s[:, :ntiles * P],
                        data_ap=u1[:, g * 8:g * 8 + 8],
                        idxs_ap=x1[:, g, :],
                        channels=P, num_elems=ntiles * P, num_idxs=8)
                    if lower:
                        u23g = u23pool.tile([P, 16], vdt, tag="u23g")
                        nc.vector.tensor_tensor(
                            out=u23g[:, 0:8],
                            in0=m[:, g * 8:g * 8 + 8, MC_WB],
                            in1=m[:, g * 8:g * 8 + 8, MC_OP2], op=MUL)
                        nc.vector.tensor_tensor(
                            out=u23g[:, 8:16],
                            in0=m[:, g * 8:g * 8 + 8, MC_WC],
                            in1=m[:, g * 8:g * 8 + 8, MC_OP2], op=MUL)
                        o23s = opool.tile([P, 8 * P], vdt, tag="o23s")
                        nc.gpsimd.local_scatter(
                            out_ap=o23s[:], data_ap=u23g[:],
                            idxs_ap=x23[:, g, :],
                            channels=P, num_elems=8 * P, num_idxs=16)
                    ps23 = None
                    for tloc in range(ntiles):
                        v = vtile(_tile_of(g, tloc))
                        nc.tensor.matmul(
                            out=ps1[:], lhsT=o1s[:, tloc * P:(tloc + 1) * P],
                            rhs=v, start=(tloc == 0), stop=(tloc == ntiles - 1))
                        if lower:
                            if tloc % 2 == 0:
                                ps23 = ps23pool.tile([P, F], fp32, tag="ps23")
                            nc.tensor.matmul(
                                out=ps23[:],
                                lhsT=o23s[:, tloc * P:(tloc + 1) * P],
                                rhs=v, start=(tloc % 2 == 0),
                                stop=(tloc % 2 == 1))
                            if tloc % 2 == 1:
                                k = 4 * g + tloc // 2
                                nc.scalar.copy(
                                    out=outreg[:, k * P:(k + 1) * P],
                                    in_=ps23[:])
                    # car/cdr drain: bins [128g, +128) add onto cons copy
                    nc.vector.tensor_tensor(
                        out=outreg[:, g * P:(g + 1) * P],
                        in0=outreg[:, g * P:(g + 1) * P], in1=ps1[:], op=ADD)
                nc.sync.dma_start(
                    out=out[b].rearrange("(k p) f -> p k f", p=P),
                    in_=outreg[:].rearrange("p (k f) -> p k f", f=F))

    nc.compile()
    return nc


def _pack_inputs(mem_values, arg_weights, root_filler, op_dist,
                 batch_idx, slot_idx, role_idx):
    """Host-side sharding/packing. Index selection and copies only."""
    mem_values = np.ascontiguousarray(mem_values, dtype=np.float32)
    arg_weights = np.asarray(arg_weights, dtype=np.float32)
    root_filler = np.asarray(root_filler, dtype=np.float32)
    op_dist = np.asarray(op_dist, dtype=np.float32)
    batch_idx = np.asarray(batch_idx, dtype=np.int64)
    slot_idx = np.asarray(slot_idx, dtype=np.int64)
    role_idx = np.asarray(role_idx, dtype=np.int64)

    # per-entry selected copies (pure gathers, no arithmetic)
    w = arg_weights[batch_idx, slot_idx]  # [N, 4] copies
    r = role_idx
    even = (r & 1) == 0
    wA = np.where(even, w[:, 0], np.where(r != 1, w[:, 1], 0.0)).astype(np.float32)
    opA = np.where(even, op_dist[batch_idx, 0],
                   op_dist[batch_idx, 1]).astype(np.float32)
    lo = r < H
    wB = np.where(lo, w[:, 2], 0.0).astype(np.float32)
    wC = np.where(lo, w[:, 3], 0.0).astype(np.float32)
    op2c = op_dist[batch_idx, 2].astype(np.float32)

    # block id within batch: lower cons blocks 0..31 (64 r each),
    # upper blocks 32..39 (256 r each)
    blk = np.where(lo, r >> 6, 32 + ((r - H) >> 8))
    cap_slots = np.concatenate([
        np.full(32, LOW_CAP * P, np.int64), np.full(8, UP_CAP * P, np.int64)])
    blk_slot0 = np.concatenate([[0], np.cumsum(cap_slots)])[:-1]  # [40]

    vdt = np.dtype(CONFIG["val_dtype"])
    in_maps = []
    for c in range(NCORES):
        vals_s = np.zeros((NT * P, F), vdt)
        # entry-indexed (tile space) scratch, converted to slot space below
        r1_rel = np.full((NT, P), -1, np.int64)
        r23_rel = np.full((NT, P), -1, np.int64)
        wA_t = np.zeros((NT, P), np.float32)
        opA_t = np.zeros((NT, P), np.float32)
        wB_t = np.zeros((NT, P), np.float32)
        wC_t = np.zeros((NT, P), np.float32)
        op2_t = np.zeros((NT, P), np.float32)
        for bb in range(BPC):
            b = c * BPC + bb
            sel = np.nonzero(batch_idx == b)[0]
            gb = blk[sel]
            order = np.argsort(gb, kind="stable")
            sel = sel[order]
            gb = gb[order]
            counts = np.bincount(gb, minlength=40)
            if (counts[:32] > LOW_CAP * P - 1).any() or \
               (counts[32:] > UP_CAP * P).any():
                raise RuntimeError(
                    "static schedule capacity exceeded: "
                    f"lower={counts[:32].max()} upper={counts[32:].max()}")
            first = np.concatenate([[0], np.cumsum(counts)])[:-1]
            pos = np.arange(sel.size) - first[gb]
            slot = blk_slot0[gb] + pos + bb * TILES_PER_BATCH * P
            vals_s[slot] = mem_values[sel]
            tix, pix = slot // P, slot % P
            rr = role_idx[sel]
            r1_rel[tix, pix] = (rr >> 1) & 127
            r23_rel[tix, pix] = np.where(rr < H, rr & 63, -1)
            wA_t[tix, pix] = wA[sel]
            opA_t[tix, pix] = opA[sel]
            wB_t[tix, pix] = wB[sel]
            wC_t[tix, pix] = wC[sel]
            op2_t[tix, pix] = op2c[sel]
            # synthetic root entry -> bin 1 == 2*0+1 (block 0, odd cons)
            rslot = bb * TILES_PER_BATCH * P + counts[0]
            vals_s[rslot] = root_filler[b]
            ti, pi = rslot // P, rslot % P
            r1_rel[ti, pi] = -1
            r23_rel[ti, pi] = 0
            wC_t[ti, pi] = 1.0
            op2_t[ti, pi] = op_dist[b, 2]

        # tile space -> slot space
        meta_s = np.zeros((BPC, NSLOT, P, NMC), np.float32)
        idx1_s = np.full((BPC, NG, P, 8), -1, np.int16)
        idx23_s = np.full((BPC, 8, P, 16), -1, np.int16)
        for bb in range(BPC):
            for g in range(NG):
                ntl = 8 if g < 8 else UP_CAP
                for tloc in range(ntl):
                    t = bb * TILES_PER_BATCH + _tile_of(g, tloc)
                    s = _slot_of(g, tloc)
                    meta_s[bb, s, :, MC_WA] = wA_t[t]
                    meta_s[bb, s, :, MC_OPA] = opA_t[t]
                    meta_s[bb, s, :, MC_WB] = wB_t[t]
                    meta_s[bb, s, :, MC_WC] = wC_t[t]
                    meta_s[bb, s, :, MC_OP2] = op2_t[t]
                    meta_s[bb, s, :, MC_R1] = r1_rel[t]
                    meta_s[bb, s, :, MC_R23] = r23_rel[t]
                    v1 = r1_rel[t] >= 0
                    idx1_s[bb, g, :, tloc] = np.where(
                        v1, tloc * P + r1_rel[t], -1)
                    if g < 8:
                        v23 = r23_rel[t] >= 0
                        base = tloc * P + 2 * r23_rel[t]
                        idx23_s[bb, g, :, tloc] = np.where(v23, base, -1)
                        idx23_s[bb, g, :, 8 + tloc] = np.where(v23, base + 1, -1)

        in_maps.append({
            "vals": vals_s,
            "meta": meta_s,
            "idx1": idx1_s,
            "idx23": idx23_s,
        })
    return in_maps


def kernel(**inputs):
    from concourse.bass_utils import run_bass_kernel_spmd

    in_maps = _pack_inputs(**inputs)
    if "nc" not in _PROG_CACHE:
        _PROG_CACHE["nc"] = _build_program()
    nc = _PROG_CACHE["nc"]
    res = run_bass_kernel_spmd(nc, in_maps, list(range(NCORES)))
    return np.concatenate([res.results[c]["out"] for c in range(NCORES)], axis=0)


# revision 16
# speedup vs baseline: 6.1836x; 1.0926x over previous
"""DiffTreeInterpreter scatter-coalesce kernel for 8 Trainium2 cores.

Data-parallel over batch B=32: core c owns batches [4c, 4c+4). All
scatter-adds are device-local. Host work is limited to sharding-style
index prep: bucketing entries by (batch, role-block), and shipping
bit-exact *copies* of per-entry weights (arg_weights / op_dist rows
selected by index) alongside the value stream. All arithmetic
(weight products, value scaling, coalesce sums, stream combine)
happens on the NeuronCores.

Math (see reference): with H = R/2, each entry n (b, l, r, v=mem[n],
w=arg_weights[b,l]) contributes to out[b] at up to 3 bins:
  bin r>>1   with weight op0[b]*w0 if r even, op1[b]*w1 if r odd and r!=1
  bin 2r     with weight op2[b]*w2 (only r < H)
  bin 2r+1   with weight op2[b]*w3 (only r < H)
plus out[b,1] += op2[b]*root_filler[b].
(The reference's pad-mask is a no-op on values: masked rows are all-zero.)

Device algorithm per core: entries are bucketed into 128-entry tiles
aligned to role windows; tiles are organized into 16 groups per batch
(8 "lower" groups of 8 tiles covering r<2048, feeding both the
car/cdr stream and the interleaved cons stream; 8 "upper" groups of
5 tiles covering r>=2048, car/cdr only). Per group, GPSIMD
local_scatter builds u-scaled one-hot slabs in fp16 (u = weight
products computed on the Vector engine); the PE contracts one-hot^T @
values into PSUM blocks of 128 output bins; PSUM drains into a
per-batch SBUF output region (ACT copies + DVE adds) which is written
out with one DMA per batch.
"""

import sys

if "/opt/trn_rl_repo" not in sys.path:
    sys.path.insert(0, "/opt/trn_rl_repo")

import numpy as np

B, L, F, R = 32, 128, 128, 4096
H = R >> 1
N = 262144
NCORES = 8
BPC = B // NCORES  # batches per core

P = 128  # partitions / tile entry count / bin-block size

# Static schedule per batch: 16 groups; lower groups g<8 have 8 tiles
# (4 cons blocks x 2 tiles, r in [256g, 256g+256)); upper groups 5 tiles.
NG = 16
LOW_CAP = 2   # tiles per (batch, 64-r block); holds <= 256 entries
UP_CAP = 5    # tiles per (batch, 256-r block); holds <= 640 entries
TILES_PER_BATCH = 32 * LOW_CAP + 8 * UP_CAP  # 104
NSLOT = NG * 8  # group-padded slot space (upper groups use 5 of 8)
NT = BPC * TILES_PER_BATCH  # tiles per core (416)

# meta channels (fp32, slot space)
MC_WA, MC_OPA, MC_WB, MC_WC, MC_OP2, MC_R1, MC_R23, MC_PAD = range(8)
NMC = 8

_PROG_CACHE = {}

CONFIG = {
    "val_dtype": "float16",  # PE operand dtype (values + one-hots)
    "vload_batch": 8,        # value tiles per load DMA
}


def _slot_of(g, tloc):
    return g * 8 + tloc


def _tile_of(g, tloc):
    if g < 8:
        return g * 8 + tloc
    return 64 + (g - 8) * UP_CAP + tloc


def _build_program():
    import concourse.bacc as bacc
    import concourse.mybir as mybir
    import concourse.tile as tile

    fp32 = mybir.dt.float32
    i16 = mybir.dt.int16
    vdt = getattr(mybir.dt, CONFIG["val_dtype"])
    MUL = mybir.AluOpType.mult
    ADD = mybir.AluOpType.add
    VB = CONFIG["vload_batch"]
    assert TILES_PER_BATCH % VB == 0

    nc = bacc.Bacc(None, target_bir_lowering=False)
    # values grouped by load-slab: [group, partition, tile-in-group, F] so
    # each partition's DMA read is VB*F contiguous elements
    vals = nc.dram_tensor("vals", [NT // VB, P, VB, F], vdt,
                          kind="ExternalInput")
    meta = nc.dram_tensor("meta", [BPC, NSLOT, P, NMC], fp32,
                          kind="ExternalInput")
    idx1 = nc.dram_tensor("idx1", [BPC, NG, P, 8], i16, kind="ExternalInput")
    idx23 = nc.dram_tensor("idx23", [BPC, 8, P, 16], i16, kind="ExternalInput")
    out = nc.dram_tensor("out", [BPC, R, F], fp32, kind="ExternalOutput")

    with tile.TileContext(nc) as tc:
        with tc.tile_pool(name="metap", bufs=BPC) as mpool, \
             tc.tile_pool(name="useq", bufs=BPC) as upool, \
             tc.tile_pool(name="u23p", bufs=8) as u23pool, \
             tc.tile_pool(name="vload", bufs=8) as vpool, \
             tc.tile_pool(name="ohot", bufs=8) as opool, \
             tc.tile_pool(name="outreg", bufs=2) as rpool, \
             tc.tile_pool(name="ps1", bufs=3, space="PSUM") as ps1pool, \
             tc.tile_pool(name="ps23", bufs=4, space="PSUM") as ps23pool:

            vtiles = {}

            # prefetch all batches' metadata up front (small, keeps the
            # batch-transition critical path off the DMA queue)
            metas = []
            for b in range(BPC):
                m = mpool.tile([P, NSLOT, NMC], fp32, tag="m")
                nc.sync.dma_start(
                    out=m[:], in_=meta[b].rearrange("s p c -> p s c"))
                x1 = mpool.tile([P, NG, 8], i16, tag="x1")
                nc.sync.dma_start(
                    out=x1[:], in_=idx1[b].rearrange("g p s -> p g s"))
                x23 = mpool.tile([P, 8, 16], i16, tag="x23")
                nc.sync.dma_start(
                    out=x23[:], in_=idx23[b].rearrange("g p s -> p g s"))
                u1 = upool.tile([P, NSLOT], vdt, tag="u1")
                nc.vector.tensor_tensor(
                    out=u1[:], in0=m[:, :, MC_WA], in1=m[:, :, MC_OPA], op=MUL)
                metas.append((m, x1, x23, u1))

            for b in range(BPC):
                m, x1, x23, u1 = metas[b]
                outreg = rpool.tile([P, 32 * P], fp32)

                def vtile(t):
                    tg = b * TILES_PER_BATCH + t
                    g = tg // VB
                    if g not in vtiles:
                        vt = vpool.tile([P, VB, F], vdt, tag="v")
                        nc.sync.dma_start(out=vt[:], in_=vals[g])
                        vtiles[g] = vt
                    return vtiles[g][:, tg % VB, :]

                for g in range(NG):
                    lower = g < 8
                    ntiles = 8 if lower else UP_CAP
                    ps1 = ps1pool.tile([P, F], fp32, tag="ps1")
                    # group one-hot slabs via GPSIMD local scatter
                    o1s = opool.tile([P, 8 * P], vdt, tag="o1s")
                    nc.gpsimd.local_scatter(
                        out_ap=o1s[:, :ntiles * P],
                        data_ap=u1[:, g * 8:g * 8 + 8],
                        idxs_ap=x1[:, g, :],
                        channels=P, num_elems=ntiles * P, num_idxs=8)
                    if lower:
                        u23g = u23pool.tile([P, 16], vdt, tag="u23g")
                        nc.vector.tensor_tensor(
                            out=u23g[:, 0:8],
                            in0=m[:, g * 8:g * 8 + 8, MC_WB],
                            in1=m[:, g * 8:g * 8 + 8, MC_OP2], op=MUL)
                        nc.vector.tensor_tensor(
                            out=u23g[:, 8:16],
                            in0=m[:, g * 8:g * 8 + 8, MC_WC],
                            in1=m[:, g * 8:g * 8 + 8, MC_OP2], op=MUL)
                        o23s = opool.tile([P, 8 * P], vdt, tag="o23s")
                        nc.gpsimd.local_scatter(
                            out_ap=o23s[:], data_ap=u23g[:],
                            idxs_ap=x23[:, g, :],
                            channels=P, num_elems=8 * P, num_idxs=16)
                    ps23 = None
                    for tloc in range(ntiles):
                        v = vtile(_tile_of(g, tloc))
                        nc.tensor.matmul(
                            out=ps1[:], lhsT=o1s[:, tloc * P:(tloc + 1) * P],
                            rhs=v, start=(tloc == 0), stop=(tloc == ntiles - 1))
                        if lower:
                            if tloc % 2 == 0:
                                ps23 = ps23pool.tile([P, F], fp32, tag="ps23")
                            nc.tensor.matmul(
                                out=ps23[:],
                                lhsT=o23s[:, tloc * P:(tloc + 1) * P],
                                rhs=v, start=(tloc % 2 == 0),
                                stop=(tloc % 2 == 1))
                            if tloc % 2 == 1:
                                k = 4 * g + tloc // 2
                                nc.scalar.copy(
                                    out=outreg[:, k * P:(k + 1) * P],
                                    in_=ps23[:])
                    # car/cdr drain: bins [128g, +128) add onto cons copy
                    nc.vector.tensor_tensor(
                        out=outreg[:, g * P:(g + 1) * P],
                        in0=outreg[:, g * P:(g + 1) * P], in1=ps1[:], op=ADD)
                nc.sync.dma_start(
                    out=out[b].rearrange("(k p) f -> p k f", p=P),
                    in_=outreg[:].rearrange("p (k f) -> p k f", f=F))

    nc.compile()
    return nc


def _pack_inputs(mem_values, arg_weights, root_filler, op_dist,
                 batch_idx, slot_idx, role_idx):
    """Host-side sharding/packing. Index selection and copies only."""
    mem_values = np.ascontiguousarray(mem_values, dtype=np.float32)
    arg_weights = np.asarray(arg_weights, dtype=np.float32)
    root_filler = np.asarray(root_filler, dtype=np.float32)
    op_dist = np.asarray(op_dist, dtype=np.float32)
    batch_idx = np.asarray(batch_idx, dtype=np.int64)
    slot_idx = np.asarray(slot_idx, dtype=np.int64)
    role_idx = np.asarray(role_idx, dtype=np.int64)

    # per-entry selected copies (pure gathers, no arithmetic)
    w = arg_weights[batch_idx, slot_idx]  # [N, 4] copies
    r = role_idx
    even = (r & 1) == 0
    wA = np.where(even, w[:, 0], np.where(r != 1, w[:, 1], 0.0)).astype(np.float32)
    opA = np.where(even, op_dist[batch_idx, 0],
                   op_dist[batch_idx, 1]).astype(np.float32)
    lo = r < H
    wB = np.where(lo, w[:, 2], 0.0).astype(np.float32)
    wC = np.where(lo, w[:, 3], 0.0).astype(np.float32)
    op2c = op_dist[batch_idx, 2].astype(np.float32)

    # block id within batch: lower cons blocks 0..31 (64 r each),
    # upper blocks 32..39 (256 r each)
    blk = np.where(lo, r >> 6, 32 + ((r - H) >> 8))
    cap_slots = np.concatenate([
        np.full(32, LOW_CAP * P, np.int64), np.full(8, UP_CAP * P, np.int64)])
    blk_slot0 = np.concatenate([[0], np.cumsum(cap_slots)])[:-1]  # [40]

    vdt = np.dtype(CONFIG["val_dtype"])
    VB = CONFIG["vload_batch"]
    in_maps = []
    for c in range(NCORES):
        vals_s = np.zeros((NT * P, F), vdt)
        # entry-indexed (tile space) scratch, converted to slot space below
        r1_rel = np.full((NT, P), -1, np.int64)
        r23_rel = np.full((NT, P), -1, np.int64)
        wA_t = np.zeros((NT, P), np.float32)
        opA_t = np.zeros((NT, P), np.float32)
        wB_t = np.zeros((NT, P), np.float32)
        wC_t = np.zeros((NT, P), np.float32)
        op2_t = np.zeros((NT, P), np.float32)
        for bb in range(BPC):
            b = c * BPC + bb
            sel = np.nonzero(batch_idx == b)[0]
            gb = blk[sel]
            order = np.argsort(gb, kind="stable")
            sel = sel[order]
            gb = gb[order]
            counts = np.bincount(gb, minlength=40)
            if (counts[:32] > LOW_CAP * P - 1).any() or \
               (counts[32:] > UP_CAP * P).any():
                raise RuntimeError(
                    "static schedule capacity exceeded: "
                    f"lower={counts[:32].max()} upper={counts[32:].max()}")
            first = np.concatenate([[0], np.cumsum(counts)])[:-1]
            pos = np.arange(sel.size) - first[gb]
            slot = blk_slot0[gb] + pos + bb * TILES_PER_BATCH * P
            vals_s[slot] = mem_values[sel]
            tix, pix = slot // P, slot % P
            rr = role_idx[sel]
            r1_rel[tix, pix] = (rr >> 1) & 127
            r23_rel[tix, pix] = np.where(rr < H, rr & 63, -1)
            wA_t[tix, pix] = wA[sel]
            opA_t[tix, pix] = opA[sel]
            wB_t[tix, pix] = wB[sel]
            wC_t[tix, pix] = wC[sel]
            op2_t[tix, pix] = op2c[sel]
            # synthetic root entry -> bin 1 == 2*0+1 (block 0, odd cons)
            rslot = bb * TILES_PER_BATCH * P + counts[0]
            vals_s[rslot] = root_filler[b]
            ti, pi = rslot // P, rslot % P
            r1_rel[ti, pi] = -1
            r23_rel[ti, pi] = 0
            wC_t[ti, pi] = 1.0
            op2_t[ti, pi] = op_dist[b, 2]

        # tile space -> slot space
        meta_s = np.zeros((BPC, NSLOT, P, NMC), np.float32)
        idx1_s = np.full((BPC, NG, P, 8), -1, np.int16)
        idx23_s = np.full((BPC, 8, P, 16), -1, np.int16)
        for bb in range(BPC):
            for g in range(NG):
                ntl = 8 if g < 8 else UP_CAP
                for tloc in range(ntl):
                    t = bb * TILES_PER_BATCH + _tile_of(g, tloc)
                    s = _slot_of(g, tloc)
                    meta_s[bb, s, :, MC_WA] = wA_t[t]
                    meta_s[bb, s, :, MC_OPA] = opA_t[t]
                    meta_s[bb, s, :, MC_WB] = wB_t[t]
                    meta_s[bb, s, :, MC_WC] = wC_t[t]
                    meta_s[bb, s, :, MC_OP2] = op2_t[t]
                    meta_s[bb, s, :, MC_R1] = r1_rel[t]
                    meta_s[bb, s, :, MC_R23] = r23_rel[t]
                    v1 = r1_rel[t] >= 0
                    idx1_s[bb, g, :, tloc] = np.where(
                        v1, tloc * P + r1_rel[t], -1)
                    if g < 8:
                        v23 = r23_rel[t] >= 0
                        base = tloc * P + 2 * r23_rel[t]
                        idx23_s[bb, g, :, tloc] = np.where(v23, base, -1)
                        idx23_s[bb, g, :, 8 + tloc] = np.where(v23, base + 1, -1)

        in_maps.append({
            # [NT*P, F] -> [NT//VB, P, VB, F] load-grouped layout
            "vals": np.ascontiguousarray(
                vals_s.reshape(NT // VB, VB, P, F).transpose(0, 2, 1, 3)),
            "meta": meta_s,
            "idx1": idx1_s,
            "idx23": idx23_s,
        })
    return in_maps


def kernel(**inputs):
    from concourse.bass_utils import run_bass_kernel_spmd

    in_maps = _pack_inputs(**inputs)
    if "nc" not in _PROG_CACHE:
        _PROG_CACHE["nc"] = _build_program()
    nc = _PROG_CACHE["nc"]
    res = run_bass_kernel_spmd(nc, in_maps, list(range(NCORES)))
    return np.concatenate([res.results[c]["out"] for c in range(NCORES)], axis=0)


# revision 24
# speedup vs baseline: 8.3518x; 1.3507x over previous
"""DiffTreeInterpreter scatter-coalesce kernel for 8 Trainium2 cores.

Data-parallel over batch B=32: core c owns batches [4c, 4c+4). All
scatter-adds are device-local. Host work is limited to sharding-style
index prep: bucketing entries by (batch, role-block), and shipping
bit-exact *copies* of per-entry weights (arg_weights / op_dist rows
selected by index) alongside the value stream. All arithmetic
(weight products, value scaling, coalesce sums, stream combine)
happens on the NeuronCores.

Math (see reference): with H = R/2, each entry n (b, l, r, v=mem[n],
w=arg_weights[b,l]) contributes to out[b] at up to 3 bins:
  bin r>>1   with weight op0[b]*w0 if r even, op1[b]*w1 if r odd and r!=1
  bin 2r     with weight op2[b]*w2 (only r < H)
  bin 2r+1   with weight op2[b]*w3 (only r < H)
plus out[b,1] += op2[b]*root_filler[b].
(The reference's pad-mask is a no-op on values: masked rows are all-zero.)

Device algorithm per core: entries are bucketed into 128-entry tiles
aligned to role windows; tiles are organized into 16 groups per batch
(8 "lower" groups of 8 tiles covering r<2048, feeding both the
car/cdr stream and the interleaved cons stream; 8 "upper" groups of
5 tiles covering r>=2048, car/cdr only). Per group, GPSIMD
local_scatter builds u-scaled one-hot slabs in fp16 (u = weight
products computed on the Vector engine); the PE contracts one-hot^T @
values into PSUM blocks of 128 output bins; PSUM drains into a
per-batch SBUF output region (ACT copies + DVE adds) which is written
out with one DMA per batch.
"""

import sys

if "/opt/trn_rl_repo" not in sys.path:
    sys.path.insert(0, "/opt/trn_rl_repo")

import numpy as np

B, L, F, R = 32, 128, 128, 4096
H = R >> 1
N = 262144
NCORES = 8
BPC = B // NCORES  # batches per core

P = 128  # partitions / tile entry count / bin-block size

# Static schedule per batch: 16 groups; lower groups g<8 have 8 tiles
# (4 cons blocks x 2 tiles, r in [256g, 256g+256)); upper groups 5 tiles.
NG = 16
LOW_CAP = 2   # tiles per (batch, 64-r block); holds <= 256 entries
UP_CAP = 5    # tiles per (batch, 256-r block); holds <= 640 entries
TILES_PER_BATCH = 32 * LOW_CAP + 8 * UP_CAP  # 104
NSLOT = NG * 8  # group-padded slot space (upper groups use 5 of 8)
NT = BPC * TILES_PER_BATCH  # tiles per core (416)

# meta channels (fp32, slot space)
MC_WA, MC_OPA, MC_WB, MC_WC, MC_OP2, MC_R1, MC_R23, MC_PAD = range(8)
NMC = 8

_PROG_CACHE = {}

CONFIG = {
    "val_dtype": "float16",  # PE operand dtype (values + one-hots)
    "vload_batch": 8,        # value tiles per load DMA
}


def _slot_of(g, tloc):
    return g * 8 + tloc


def _tile_of(g, tloc):
    if g < 8:
        return g * 8 + tloc
    return 64 + (g - 8) * UP_CAP + tloc


def _build_program():
    import concourse.bacc as bacc
    import concourse.mybir as mybir
    import concourse.tile as tile

    fp32 = mybir.dt.float32
    i16 = mybir.dt.int16
    vdt = getattr(mybir.dt, CONFIG["val_dtype"])
    MUL = mybir.AluOpType.mult
    ADD = mybir.AluOpType.add
    EQ = mybir.AluOpType.is_equal
    VB = CONFIG["vload_batch"]
    assert TILES_PER_BATCH % VB == 0

    nc = bacc.Bacc(None, target_bir_lowering=False)
    # values grouped by load-slab: [group, partition, tile-in-group, F] so
    # each partition's DMA read is VB*F contiguous elements
    vals = nc.dram_tensor("vals", [NT // VB, P, VB, F], vdt,
                          kind="ExternalInput")
    meta = nc.dram_tensor("meta", [BPC, P, NSLOT, NMC], fp32,
                          kind="ExternalInput")
    idx1 = nc.dram_tensor("idx1", [BPC, P, NG, 8], i16, kind="ExternalInput")
    idx23 = nc.dram_tensor("idx23", [BPC, P, 8, 16], i16, kind="ExternalInput")
    iota = nc.dram_tensor("iota", [P, P], fp32, kind="ExternalInput")
    out = nc.dram_tensor("out", [BPC, R, F], fp32, kind="ExternalOutput")

    with tile.TileContext(nc) as tc:
        with tc.tile_pool(name="metap", bufs=BPC) as mpool, \
             tc.tile_pool(name="useq", bufs=BPC) as upool, \
             tc.tile_pool(name="u23p", bufs=8) as u23pool, \
             tc.tile_pool(name="vload", bufs=8) as vpool, \
             tc.tile_pool(name="ohot", bufs=8) as opool, \
             tc.tile_pool(name="outreg", bufs=2) as rpool, \
             tc.tile_pool(name="ps1", bufs=3, space="PSUM") as ps1pool, \
             tc.tile_pool(name="ps23", bufs=4, space="PSUM") as ps23pool:

            vtiles = {}

            io_t = mpool.tile([P, P], fp32, tag="iota")
            nc.sync.dma_start(out=io_t[:], in_=iota[:])

            # prefetch all batches' metadata up front (small, keeps the
            # batch-transition critical path off the DMA queue)
            metas = []
            for b in range(BPC):
                m = mpool.tile([P, NSLOT, NMC], fp32, tag="m")
                nc.sync.dma_start(out=m[:], in_=meta[b])
                x1 = mpool.tile([P, NG, 8], i16, tag="x1")
                nc.sync.dma_start(out=x1[:], in_=idx1[b])
                x23 = mpool.tile([P, 8, 16], i16, tag="x23")
                nc.sync.dma_start(out=x23[:], in_=idx23[b])
                u1 = upool.tile([P, NSLOT], vdt, tag="u1")
                nc.vector.tensor_tensor(
                    out=u1[:], in0=m[:, :, MC_WA], in1=m[:, :, MC_OPA], op=MUL)
                u1f = upool.tile([P, NSLOT], fp32, tag="u1f")
                nc.vector.tensor_tensor(
                    out=u1f[:], in0=m[:, :, MC_WA], in1=m[:, :, MC_OPA], op=MUL)
                metas.append((m, x1, x23, u1, u1f))

            for b in range(BPC):
                m, x1, x23, u1, u1f = metas[b]
                outreg = rpool.tile([P, 32 * P], fp32)

                def vtile(t):
                    tg = b * TILES_PER_BATCH + t
                    g = tg // VB
                    if g not in vtiles:
                        vt = vpool.tile([P, VB, F], vdt, tag="v")
                        nc.sync.dma_start(out=vt[:], in_=vals[g])
                        vtiles[g] = vt
                    return vtiles[g][:, tg % VB, :]

                for g in range(NG):
                    lower = g < 8
                    ntiles = 8 if lower else UP_CAP
                    ps1 = ps1pool.tile([P, F], fp32, tag="ps1")
                    # group one-hot slabs: GPSIMD local scatter, except a
                    # share of upper groups built per-tile on the Vector
                    # engine to balance the two
                    o1s = opool.tile([P, 8 * P], vdt, tag="o1s")
                    if lower or g % 2 == 0:
                        nc.gpsimd.local_scatter(
                            out_ap=o1s[:, :ntiles * P],
                            data_ap=u1[:, g * 8:g * 8 + 8],
                            idxs_ap=x1[:, g, :],
                            channels=P, num_elems=ntiles * P, num_idxs=8)
                    else:
                        for tloc in range(ntiles):
                            s = g * 8 + tloc
                            nc.vector.tensor_scalar(
                                out=o1s[:, tloc * P:(tloc + 1) * P],
                                in0=io_t[:],
                                scalar1=m[:, s, MC_R1:MC_R1 + 1],
                                scalar2=u1f[:, s:s + 1],
                                op0=EQ, op1=MUL)
                    if lower:
                        u23g = u23pool.tile([P, 16], vdt, tag="u23g")
                        nc.vector.tensor_tensor(
                            out=u23g[:, 0:8],
                            in0=m[:, g * 8:g * 8 + 8, MC_WB],
                            in1=m[:, g * 8:g * 8 + 8, MC_OP2], op=MUL)
                        nc.vector.tensor_tensor(
                            out=u23g[:, 8:16],
                            in0=m[:, g * 8:g * 8 + 8, MC_WC],
                            in1=m[:, g * 8:g * 8 + 8, MC_OP2], op=MUL)
                        o23s = opool.tile([P, 8 * P], vdt, tag="o23s")
                        nc.gpsimd.local_scatter(
                            out_ap=o23s[:], data_ap=u23g[:],
                            idxs_ap=x23[:, g, :],
                            channels=P, num_elems=8 * P, num_idxs=16)
                    ps23 = None
                    for tloc in range(ntiles):
                        v = vtile(_tile_of(g, tloc))
                        nc.tensor.matmul(
                            out=ps1[:], lhsT=o1s[:, tloc * P:(tloc + 1) * P],
                            rhs=v, start=(tloc == 0), stop=(tloc == ntiles - 1))
                        if lower:
                            if tloc % 2 == 0:
                                ps23 = ps23pool.tile([P, F], fp32, tag="ps23")
                            nc.tensor.matmul(
                                out=ps23[:],
                                lhsT=o23s[:, tloc * P:(tloc + 1) * P],
                                rhs=v, start=(tloc % 2 == 0),
                                stop=(tloc % 2 == 1))
                            if tloc % 2 == 1:
                                k = 4 * g + tloc // 2
                                nc.scalar.copy(
                                    out=outreg[:, k * P:(k + 1) * P],
                                    in_=ps23[:])
                    # car/cdr drain: bins [128g, +128) add onto cons copy
                    nc.vector.tensor_tensor(
                        out=outreg[:, g * P:(g + 1) * P],
                        in0=outreg[:, g * P:(g + 1) * P], in1=ps1[:], op=ADD)
                nc.sync.dma_start(
                    out=out[b].rearrange("(k p) f -> p k f", p=P),
                    in_=outreg[:].rearrange("p (k f) -> p k f", f=F))

    nc.compile()
    return nc


def _pack_inputs(mem_values, arg_weights, root_filler, op_dist,
                 batch_idx, slot_idx, role_idx):
    """Host-side sharding/packing. Index selection and copies only."""
    mem_values = np.ascontiguousarray(mem_values, dtype=np.float32)
    arg_weights = np.asarray(arg_weights, dtype=np.float32)
    root_filler = np.asarray(root_filler, dtype=np.float32)
    op_dist = np.asarray(op_dist, dtype=np.float32)
    batch_idx = np.asarray(batch_idx, dtype=np.int64)
    slot_idx = np.asarray(slot_idx, dtype=np.int64)
    role_idx = np.asarray(role_idx, dtype=np.int64)

    # per-entry selected copies (pure gathers, no arithmetic)
    w = arg_weights[batch_idx, slot_idx]  # [N, 4] copies
    r = role_idx
    even = (r & 1) == 0
    wA = np.where(even, w[:, 0], np.where(r != 1, w[:, 1], 0.0)).astype(np.float32)
    opA = np.where(even, op_dist[batch_idx, 0],
                   op_dist[batch_idx, 1]).astype(np.float32)
    lo = r < H
    wB = np.where(lo, w[:, 2], 0.0).astype(np.float32)
    wC = np.where(lo, w[:, 3], 0.0).astype(np.float32)
    op2c = op_dist[batch_idx, 2].astype(np.float32)

    # block id within batch: lower cons blocks 0..31 (64 r each),
    # upper blocks 32..39 (256 r each)
    blk = np.where(lo, r >> 6, 32 + ((r - H) >> 8))
    cap_slots = np.concatenate([
        np.full(32, LOW_CAP * P, np.int64), np.full(8, UP_CAP * P, np.int64)])
    blk_slot0 = np.concatenate([[0], np.cumsum(cap_slots)])[:-1]  # [40]

    vdt = np.dtype(CONFIG["val_dtype"])
    VB = CONFIG["vload_batch"]
    in_maps = []
    for c in range(NCORES):
        vals_s = np.zeros((NT * P, F), vdt)
        # entry-indexed (tile space) scratch, converted to slot space below
        r1_rel = np.full((NT, P), -1, np.int64)
        r23_rel = np.full((NT, P), -1, np.int64)
        wA_t = np.zeros((NT, P), np.float32)
        opA_t = np.zeros((NT, P), np.float32)
        wB_t = np.zeros((NT, P), np.float32)
        wC_t = np.zeros((NT, P), np.float32)
        op2_t = np.zeros((NT, P), np.float32)
        for bb in range(BPC):
            b = c * BPC + bb
            sel = np.nonzero(batch_idx == b)[0]
            gb = blk[sel]
            order = np.argsort(gb, kind="stable")
            sel = sel[order]
            gb = gb[order]
            counts = np.bincount(gb, minlength=40)
            if (counts[:32] > LOW_CAP * P - 1).any() or \
               (counts[32:] > UP_CAP * P).any():
                raise RuntimeError(
                    "static schedule capacity exceeded: "
                    f"lower={counts[:32].max()} upper={counts[32:].max()}")
            first = np.concatenate([[0], np.cumsum(counts)])[:-1]
            pos = np.arange(sel.size) - first[gb]
            slot = blk_slot0[gb] + pos + bb * TILES_PER_BATCH * P
            vals_s[slot] = mem_values[sel]
            tix, pix = slot // P, slot % P
            rr = role_idx[sel]
            r1_rel[tix, pix] = (rr >> 1) & 127
            r23_rel[tix, pix] = np.where(rr < H, rr & 63, -1)
            wA_t[tix, pix] = wA[sel]
            opA_t[tix, pix] = opA[sel]
            wB_t[tix, pix] = wB[sel]
            wC_t[tix, pix] = wC[sel]
            op2_t[tix, pix] = op2c[sel]
            # synthetic root entry -> bin 1 == 2*0+1 (block 0, odd cons)
            rslot = bb * TILES_PER_BATCH * P + counts[0]
            vals_s[rslot] = root_filler[b]
            ti, pi = rslot // P, rslot % P
            r1_rel[ti, pi] = -1
            r23_rel[ti, pi] = 0
            wC_t[ti, pi] = 1.0
            op2_t[ti, pi] = op_dist[b, 2]

        # tile space -> slot space
        meta_s = np.zeros((BPC, NSLOT, P, NMC), np.float32)
        idx1_s = np.full((BPC, NG, P, 8), -1, np.int16)
        idx23_s = np.full((BPC, 8, P, 16), -1, np.int16)
        for bb in range(BPC):
            for g in range(NG):
                ntl = 8 if g < 8 else UP_CAP
                for tloc in range(ntl):
                    t = bb * TILES_PER_BATCH + _tile_of(g, tloc)
                    s = _slot_of(g, tloc)
                    meta_s[bb, s, :, MC_WA] = wA_t[t]
                    meta_s[bb, s, :, MC_OPA] = opA_t[t]
                    meta_s[bb, s, :, MC_WB] = wB_t[t]
                    meta_s[bb, s, :, MC_WC] = wC_t[t]
                    meta_s[bb, s, :, MC_OP2] = op2_t[t]
                    meta_s[bb, s, :, MC_R1] = r1_rel[t]
                    meta_s[bb, s, :, MC_R23] = r23_rel[t]
                    v1 = r1_rel[t] >= 0
                    idx1_s[bb, g, :, tloc] = np.where(
                        v1, tloc * P + r1_rel[t], -1)
                    if g < 8:
                        v23 = r23_rel[t] >= 0
                        base = tloc * P + 2 * r23_rel[t]
                        idx23_s[bb, g, :, tloc] = np.where(v23, base, -1)
                        idx23_s[bb, g, :, 8 + tloc] = np.where(v23, base + 1, -1)

        in_maps.append({
            # [NT*P, F] -> [NT//VB, P, VB, F] load-grouped layout
            "vals": np.ascontiguousarray(
                vals_s.reshape(NT // VB, VB, P, F).transpose(0, 2, 1, 3)),
            # partition-major layouts so each partition's DMA is contiguous
            "meta": np.ascontiguousarray(meta_s.transpose(0, 2, 1, 3)),
            "idx1": np.ascontiguousarray(idx1_s.transpose(0, 2, 1, 3)),
            "idx23": np.ascontiguousarray(idx23_s.transpose(0, 2, 1, 3)),
            "iota": np.broadcast_to(
                np.arange(P, dtype=np.float32), (P, P)).copy(),
        })
    return in_maps


def kernel(**inputs):
    from concourse.bass_utils import run_bass_kernel_spmd

    in_maps = _pack_inputs(**inputs)
    if "nc" not in _PROG_CACHE:
        _PROG_CACHE["nc"] = _build_program()
    nc = _PROG_CACHE["nc"]
    res = run_bass_kernel_spmd(nc, in_maps, list(range(NCORES)))
    return np.concatenate([res.results[c]["out"] for c in range(NCORES)], axis=0)


# revision 27
# speedup vs baseline: 8.7793x; 1.0512x over previous
"""DiffTreeInterpreter scatter-coalesce kernel for 8 Trainium2 cores.

Data-parallel over batch B=32: core c owns batches [4c, 4c+4). All
scatter-adds are device-local. Host work is limited to sharding-style
index prep: bucketing entries by (batch, role-block), and shipping
bit-exact *copies* of per-entry weights (arg_weights / op_dist rows
selected by index) alongside the value stream. All arithmetic
(weight products, value scaling, coalesce sums, stream combine)
happens on the NeuronCores.

Math (see reference): with H = R/2, each entry n (b, l, r, v=mem[n],
w=arg_weights[b,l]) contributes to out[b] at up to 3 bins:
  bin r>>1   with weight op0[b]*w0 if r even, op1[b]*w1 if r odd and r!=1
  bin 2r     with weight op2[b]*w2 (only r < H)
  bin 2r+1   with weight op2[b]*w3 (only r < H)
plus out[b,1] += op2[b]*root_filler[b].
(The reference's pad-mask is a no-op on values: masked rows are all-zero.)

Device algorithm per core: entries are bucketed into 128-entry tiles
aligned to role windows; tiles are organized into 16 groups per batch
(8 "lower" groups of 8 tiles covering r<2048, feeding both the
car/cdr stream and the interleaved cons stream; 8 "upper" groups of
5 tiles covering r>=2048, car/cdr only). Per group, GPSIMD
local_scatter builds u-scaled one-hot slabs in fp16 (u = weight
products computed on the Vector engine); the PE contracts one-hot^T @
values into PSUM blocks of 128 output bins; PSUM drains into a
per-batch SBUF output region (ACT copies + DVE adds) which is written
out with one DMA per batch.
"""

import sys

if "/opt/trn_rl_repo" not in sys.path:
    sys.path.insert(0, "/opt/trn_rl_repo")

import numpy as np

B, L, F, R = 32, 128, 128, 4096
H = R >> 1
N = 262144
NCORES = 8
BPC = B // NCORES  # batches per core

P = 128  # partitions / tile entry count / bin-block size

# Static schedule per batch: 16 groups; lower groups g<8 have 8 tiles
# (4 cons blocks x 2 tiles, r in [256g, 256g+256)); upper groups 5 tiles.
NG = 16
LOW_CAP = 2   # tiles per (batch, 64-r block); holds <= 256 entries
UP_CAP = 5    # tiles per (batch, 256-r block); holds <= 640 entries
TILES_PER_BATCH = 32 * LOW_CAP + 8 * UP_CAP  # 104
NSLOT = NG * 8  # group-padded slot space (upper groups use 5 of 8)
NT = BPC * TILES_PER_BATCH  # tiles per core (416)

# meta channels (fp32, slot space)
MC_WA, MC_OPA, MC_WB, MC_WC, MC_OP2, MC_R1, MC_R23, MC_PAD = range(8)
NMC = 8

_PROG_CACHE = {}

CONFIG = {
    "val_dtype": "float16",  # PE operand dtype (values + one-hots)
    "vload_batch": 8,        # value tiles per load DMA
}


def _slot_of(g, tloc):
    return g * 8 + tloc


def _tile_of(g, tloc):
    if g < 8:
        return g * 8 + tloc
    return 64 + (g - 8) * UP_CAP + tloc


def _build_program():
    import concourse.bacc as bacc
    import concourse.mybir as mybir
    import concourse.tile as tile

    fp32 = mybir.dt.float32
    i16 = mybir.dt.int16
    vdt = getattr(mybir.dt, CONFIG["val_dtype"])
    MUL = mybir.AluOpType.mult
    ADD = mybir.AluOpType.add
    EQ = mybir.AluOpType.is_equal
    VB = CONFIG["vload_batch"]
    assert TILES_PER_BATCH % VB == 0

    nc = bacc.Bacc(None, target_bir_lowering=False)
    # values grouped by load-slab: [group, partition, tile-in-group, F] so
    # each partition's DMA read is VB*F contiguous elements
    vals = nc.dram_tensor("vals", [NT // VB, P, VB, F], vdt,
                          kind="ExternalInput")
    meta = nc.dram_tensor("meta", [BPC, P, NSLOT, NMC], fp32,
                          kind="ExternalInput")
    idx1 = nc.dram_tensor("idx1", [BPC, P, NG, 8], i16, kind="ExternalInput")
    idx23 = nc.dram_tensor("idx23", [BPC, P, 8, 16], i16, kind="ExternalInput")
    iota = nc.dram_tensor("iota", [P, P], fp32, kind="ExternalInput")
    out = nc.dram_tensor("out", [BPC, R, F], fp32, kind="ExternalOutput")

    with tile.TileContext(nc) as tc:
        with tc.tile_pool(name="metap", bufs=BPC) as mpool, \
             tc.tile_pool(name="useq", bufs=BPC) as upool, \
             tc.tile_pool(name="u23p", bufs=8) as u23pool, \
             tc.tile_pool(name="vload", bufs=10) as vpool, \
             tc.tile_pool(name="ohot", bufs=8) as opool, \
             tc.tile_pool(name="outreg", bufs=2) as rpool, \
             tc.tile_pool(name="ps1", bufs=4, space="PSUM") as ps1pool, \
             tc.tile_pool(name="ps23", bufs=4, space="PSUM") as ps23pool:

            vtiles = {}

            io_t = mpool.tile([P, P], fp32, tag="iota")
            nc.sync.dma_start(out=io_t[:], in_=iota[:])

            # prefetch all batches' metadata up front (small, keeps the
            # batch-transition critical path off the DMA queue)
            metas = []
            for b in range(BPC):
                m = mpool.tile([P, NSLOT, NMC], fp32, tag="m")
                nc.sync.dma_start(out=m[:], in_=meta[b])
                x1 = mpool.tile([P, NG, 8], i16, tag="x1")
                nc.sync.dma_start(out=x1[:], in_=idx1[b])
                x23 = mpool.tile([P, 8, 16], i16, tag="x23")
                nc.sync.dma_start(out=x23[:], in_=idx23[b])
                u1 = upool.tile([P, NSLOT], vdt, tag="u1")
                nc.vector.tensor_tensor(
                    out=u1[:], in0=m[:, :, MC_WA], in1=m[:, :, MC_OPA], op=MUL)
                u1f = upool.tile([P, NSLOT], fp32, tag="u1f")
                nc.vector.tensor_tensor(
                    out=u1f[:], in0=m[:, :, MC_WA], in1=m[:, :, MC_OPA], op=MUL)
                metas.append((m, x1, x23, u1, u1f))

            for b in range(BPC):
                m, x1, x23, u1, u1f = metas[b]
                outreg = rpool.tile([P, 32 * P], fp32)

                def vtile(t):
                    tg = b * TILES_PER_BATCH + t
                    g = tg // VB
                    if g not in vtiles:
                        vt = vpool.tile([P, VB, F], vdt, tag="v")
                        nc.sync.dma_start(out=vt[:], in_=vals[g])
                        vtiles[g] = vt
                    return vtiles[g][:, tg % VB, :]

                for g in range(NG):
                    lower = g < 8
                    ntiles = 8 if lower else UP_CAP
                    ps1 = ps1pool.tile([P, F], fp32, tag="ps1")
                    # group one-hot slabs: GPSIMD local scatter, except a
                    # share of upper groups built per-tile on the Vector
                    # engine to balance the two
                    o1s = opool.tile([P, 8 * P], vdt, tag="o1s")
                    if lower or g % 2 == 0:
                        nc.gpsimd.local_scatter(
                            out_ap=o1s[:, :ntiles * P],
                            data_ap=u1[:, g * 8:g * 8 + 8],
                            idxs_ap=x1[:, g, :],
                            channels=P, num_elems=ntiles * P, num_idxs=8)
                    else:
                        for tloc in range(ntiles):
                            s = g * 8 + tloc
                            nc.vector.tensor_scalar(
                                out=o1s[:, tloc * P:(tloc + 1) * P],
                                in0=io_t[:],
                                scalar1=m[:, s, MC_R1:MC_R1 + 1],
                                scalar2=u1f[:, s:s + 1],
                                op0=EQ, op1=MUL)
                    if lower:
                        u23g = u23pool.tile([P, 16], vdt, tag="u23g")
                        nc.vector.tensor_tensor(
                            out=u23g[:, 0:8],
                            in0=m[:, g * 8:g * 8 + 8, MC_WB],
                            in1=m[:, g * 8:g * 8 + 8, MC_OP2], op=MUL)
                        nc.vector.tensor_tensor(
                            out=u23g[:, 8:16],
                            in0=m[:, g * 8:g * 8 + 8, MC_WC],
                            in1=m[:, g * 8:g * 8 + 8, MC_OP2], op=MUL)
                        o23s = opool.tile([P, 8 * P], vdt, tag="o23s")
                        nc.gpsimd.local_scatter(
                            out_ap=o23s[:], data_ap=u23g[:],
                            idxs_ap=x23[:, g, :],
                            channels=P, num_elems=8 * P, num_idxs=16)
                    ps23 = None
                    for tloc in range(ntiles):
                        v = vtile(_tile_of(g, tloc))
                        nc.tensor.matmul(
                            out=ps1[:], lhsT=o1s[:, tloc * P:(tloc + 1) * P],
                            rhs=v, start=(tloc == 0), stop=(tloc == ntiles - 1))
                        if lower:
                            if tloc % 2 == 0:
                                ps23 = ps23pool.tile([P, F], fp32, tag="ps23")
                            nc.tensor.matmul(
                                out=ps23[:],
                                lhsT=o23s[:, tloc * P:(tloc + 1) * P],
                                rhs=v, start=(tloc % 2 == 0),
                                stop=(tloc % 2 == 1))
                            if tloc % 2 == 1:
                                k = 4 * g + tloc // 2
                                nc.scalar.copy(
                                    out=outreg[:, k * P:(k + 1) * P],
                                    in_=ps23[:])
                    # car/cdr drain: bins [128g, +128) add onto cons copy
                    nc.vector.tensor_tensor(
                        out=outreg[:, g * P:(g + 1) * P],
                        in0=outreg[:, g * P:(g + 1) * P], in1=ps1[:], op=ADD)

                    # flush finished bin-blocks early to shorten the tail:
                    # after g7, blocks 0-7 (car/cdr done) and 16-31 (cons
                    # only) are final; blocks 8-15 finalize at their group.
                    def flush(k0, k1):
                        nc.sync.dma_start(
                            out=out[b, k0 * P:k1 * P, :]
                            .rearrange("(k p) f -> p k f", p=P),
                            in_=outreg[:, k0 * P:k1 * P]
                            .rearrange("p (k f) -> p k f", f=F))
                    if g == 7:
                        flush(0, 8)
                        flush(16, 32)
                    elif g == 11:
                        flush(8, 12)
                    elif g == 15:
                        flush(12, 16)

    nc.compile()
    return nc


def _pack_inputs(mem_values, arg_weights, root_filler, op_dist,
                 batch_idx, slot_idx, role_idx):
    """Host-side sharding/packing. Index selection and copies only."""
    mem_values = np.ascontiguousarray(mem_values, dtype=np.float32)
    arg_weights = np.asarray(arg_weights, dtype=np.float32)
    root_filler = np.asarray(root_filler, dtype=np.float32)
    op_dist = np.asarray(op_dist, dtype=np.float32)
    batch_idx = np.asarray(batch_idx, dtype=np.int64)
    slot_idx = np.asarray(slot_idx, dtype=np.int64)
    role_idx = np.asarray(role_idx, dtype=np.int64)

    # per-entry selected copies (pure gathers, no arithmetic)
    w = arg_weights[batch_idx, slot_idx]  # [N, 4] copies
    r = role_idx
    even = (r & 1) == 0
    wA = np.where(even, w[:, 0], np.where(r != 1, w[:, 1], 0.0)).astype(np.float32)
    opA = np.where(even, op_dist[batch_idx, 0],
                   op_dist[batch_idx, 1]).astype(np.float32)
    lo = r < H
    wB = np.where(lo, w[:, 2], 0.0).astype(np.float32)
    wC = np.where(lo, w[:, 3], 0.0).astype(np.float32)
    op2c = op_dist[batch_idx, 2].astype(np.float32)

    # block id within batch: lower cons blocks 0..31 (64 r each),
    # upper blocks 32..39 (256 r each)
    blk = np.where(lo, r >> 6, 32 + ((r - H) >> 8))
    cap_slots = np.concatenate([
        np.full(32, LOW_CAP * P, np.int64), np.full(8, UP_CAP * P, np.int64)])
    blk_slot0 = np.concatenate([[0], np.cumsum(cap_slots)])[:-1]  # [40]

    vdt = np.dtype(CONFIG["val_dtype"])
    VB = CONFIG["vload_batch"]
    in_maps = []
    for c in range(NCORES):
        vals_s = np.zeros((NT * P, F), vdt)
        # entry-indexed (tile space) scratch, converted to slot space below
        r1_rel = np.full((NT, P), -1, np.int64)
        r23_rel = np.full((NT, P), -1, np.int64)
        wA_t = np.zeros((NT, P), np.float32)
        opA_t = np.zeros((NT, P), np.float32)
        wB_t = np.zeros((NT, P), np.float32)
        wC_t = np.zeros((NT, P), np.float32)
        op2_t = np.zeros((NT, P), np.float32)
        for bb in range(BPC):
            b = c * BPC + bb
            sel = np.nonzero(batch_idx == b)[0]
            gb = blk[sel]
            order = np.argsort(gb, kind="stable")
            sel = sel[order]
            gb = gb[order]
            counts = np.bincount(gb, minlength=40)
            if (counts[:32] > LOW_CAP * P - 1).any() or \
               (counts[32:] > UP_CAP * P).any():
                raise RuntimeError(
                    "static schedule capacity exceeded: "
                    f"lower={counts[:32].max()} upper={counts[32:].max()}")
            first = np.concatenate([[0], np.cumsum(counts)])[:-1]
            pos = np.arange(sel.size) - first[gb]
            slot = blk_slot0[gb] + pos + bb * TILES_PER_BATCH * P
            vals_s[slot] = mem_values[sel]
            tix, pix = slot // P, slot % P
            rr = role_idx[sel]
            r1_rel[tix, pix] = (rr >> 1) & 127
            r23_rel[tix, pix] = np.where(rr < H, rr & 63, -1)
            wA_t[tix, pix] = wA[sel]
            opA_t[tix, pix] = opA[sel]
            wB_t[tix, pix] = wB[sel]
            wC_t[tix, pix] = wC[sel]
            op2_t[tix, pix] = op2c[sel]
            # synthetic root entry -> bin 1 == 2*0+1 (block 0, odd cons)
            rslot = bb * TILES_PER_BATCH * P + counts[0]
            vals_s[rslot] = root_filler[b]
            ti, pi = rslot // P, rslot % P
            r1_rel[ti, pi] = -1
            r23_rel[ti, pi] = 0
            wC_t[ti, pi] = 1.0
            op2_t[ti, pi] = op_dist[b, 2]

        # tile space -> slot space
        meta_s = np.zeros((BPC, NSLOT, P, NMC), np.float32)
        idx1_s = np.full((BPC, NG, P, 8), -1, np.int16)
        idx23_s = np.full((BPC, 8, P, 16), -1, np.int16)
        for bb in range(BPC):
            for g in range(NG):
                ntl = 8 if g < 8 else UP_CAP
                for tloc in range(ntl):
                    t = bb * TILES_PER_BATCH + _tile_of(g, tloc)
                    s = _slot_of(g, tloc)
                    meta_s[bb, s, :, MC_WA] = wA_t[t]
                    meta_s[bb, s, :, MC_OPA] = opA_t[t]
                    meta_s[bb, s, :, MC_WB] = wB_t[t]
                    meta_s[bb, s, :, MC_WC] = wC_t[t]
                    meta_s[bb, s, :, MC_OP2] = op2_t[t]
                    meta_s[bb, s, :, MC_R1] = r1_rel[t]
                    meta_s[bb, s, :, MC_R23] = r23_rel[t]
                    v1 = r1_rel[t] >= 0
                    idx1_s[bb, g, :, tloc] = np.where(
                        v1, tloc * P + r1_rel[t], -1)
                    if g < 8:
                        v23 = r23_rel[t] >= 0
                        base = tloc * P + 2 * r23_rel[t]
                        idx23_s[bb, g, :, tloc] = np.where(v23, base, -1)
                        idx23_s[bb, g, :, 8 + tloc] = np.where(v23, base + 1, -1)

        in_maps.append({
            # [NT*P, F] -> [NT//VB, P, VB, F] load-grouped layout
            "vals": np.ascontiguousarray(
                vals_s.reshape(NT // VB, VB, P, F).transpose(0, 2, 1, 3)),
            # partition-major layouts so each partition's DMA is contiguous
            "meta": np.ascontiguousarray(meta_s.transpose(0, 2, 1, 3)),
            "idx1": np.ascontiguousarray(idx1_s.transpose(0, 2, 1, 3)),
            "idx23": np.ascontiguousarray(idx23_s.transpose(0, 2, 1, 3)),
            "iota": np.broadcast_to(
                np.arange(P, dtype=np.float32), (P, P)).copy(),
        })
    return in_maps


def kernel(**inputs):
    from concourse.bass_utils import run_bass_kernel_spmd

    in_maps = _pack_inputs(**inputs)
    if "nc" not in _PROG_CACHE:
        _PROG_CACHE["nc"] = _build_program()
    nc = _PROG_CACHE["nc"]
    res = run_bass_kernel_spmd(nc, in_maps, list(range(NCORES)))
    return np.concatenate([res.results[c]["out"] for c in range(NCORES)], axis=0)


# revision 33
# speedup vs baseline: 9.0582x; 1.0318x over previous
"""DiffTreeInterpreter scatter-coalesce kernel for 8 Trainium2 cores.

Data-parallel over batch B=32: core c owns batches [4c, 4c+4). All
scatter-adds are device-local. Host work is limited to sharding-style
index prep: bucketing entries by (batch, role-block), and shipping
bit-exact *copies* of per-entry weights (arg_weights / op_dist rows
selected by index) alongside the value stream. All arithmetic
(weight products, value scaling, coalesce sums, stream combine)
happens on the NeuronCores.

Math (see reference): with H = R/2, each entry n (b, l, r, v=mem[n],
w=arg_weights[b,l]) contributes to out[b] at up to 3 bins:
  bin r>>1   with weight op0[b]*w0 if r even, op1[b]*w1 if r odd and r!=1
  bin 2r     with weight op2[b]*w2 (only r < H)
  bin 2r+1   with weight op2[b]*w3 (only r < H)
plus out[b,1] += op2[b]*root_filler[b].
(The reference's pad-mask is a no-op on values: masked rows are all-zero.)

Device algorithm per core: entries are bucketed into 128-entry tiles
aligned to role windows; tiles are organized into 16 groups per batch
(8 "lower" groups of 8 tiles covering r<2048, feeding both the
car/cdr stream and the interleaved cons stream; 8 "upper" groups of
5 tiles covering r>=2048, car/cdr only). Per group, GPSIMD
local_scatter builds u-scaled one-hot slabs in fp16 (u = weight
products computed on the Vector engine); the PE contracts one-hot^T @
values into PSUM blocks of 128 output bins; PSUM drains into a
per-batch SBUF output region (ACT copies + DVE adds) which is written
out with one DMA per batch.
"""

import sys

if "/opt/trn_rl_repo" not in sys.path:
    sys.path.insert(0, "/opt/trn_rl_repo")

import numpy as np

B, L, F, R = 32, 128, 128, 4096
H = R >> 1
N = 262144
NCORES = 8
BPC = B // NCORES  # batches per core

P = 128  # partitions / tile entry count / bin-block size

# Static schedule per batch: 16 groups covering 256 roles each; lower
# groups g<8 (r<2048) hold 2 pairs of cons blocks, straddle-packed as
# 3 tiles per pair (T0 pure-A, T1 = A-overflow + B-overflow, T2
# pure-B); upper groups 5 tiles, car/cdr only.
NG = 16
LOW_TPG = 6   # tiles per lower group (2 pairs x 3)
UP_CAP = 5    # tiles per (batch, 256-r block); holds <= 640 entries
BLK_CAP = 256   # max entries per 64-r cons block
PAIR_CAP = 384  # max entries per cons block pair
TILES_PER_BATCH = 8 * LOW_TPG + 8 * UP_CAP  # 88
NSLOT = NG * 8  # group-padded slot space
NT = BPC * TILES_PER_BATCH  # tiles per core (352)

# meta channels (fp32, slot space)
MC_WA, MC_OPA, MC_WB, MC_WC, MC_OP2, MC_R1, MC_R23, MC_PAD = range(8)
NMC = 8

_PROG_CACHE = {}

CONFIG = {
    "val_dtype": "float16",  # PE operand dtype (values + one-hots)
    "vload_batch": 8,        # value tiles per load DMA
}


def _slot_of(g, tloc):
    return g * 8 + tloc


def _tile_of(g, tloc):
    if g < 8:
        return g * LOW_TPG + tloc
    return 8 * LOW_TPG + (g - 8) * UP_CAP + tloc


def _build_program():
    import concourse.bacc as bacc
    import concourse.mybir as mybir
    import concourse.tile as tile

    fp32 = mybir.dt.float32
    i16 = mybir.dt.int16
    vdt = getattr(mybir.dt, CONFIG["val_dtype"])
    MUL = mybir.AluOpType.mult
    ADD = mybir.AluOpType.add
    EQ = mybir.AluOpType.is_equal
    VB = CONFIG["vload_batch"]
    assert TILES_PER_BATCH % VB == 0

    nc = bacc.Bacc(None, target_bir_lowering=False)
    # values grouped by load-slab: [group, partition, tile-in-group, F] so
    # each partition's DMA read is VB*F contiguous elements
    vals = nc.dram_tensor("vals", [NT // VB, P, VB, F], vdt,
                          kind="ExternalInput")
    meta = nc.dram_tensor("meta", [BPC, P, NSLOT, NMC], fp32,
                          kind="ExternalInput")
    idx1 = nc.dram_tensor("idx1", [BPC, P, NG, 8], i16, kind="ExternalInput")
    idx23 = nc.dram_tensor("idx23", [BPC, P, 8, 16], i16, kind="ExternalInput")
    iota = nc.dram_tensor("iota", [P, P], fp32, kind="ExternalInput")
    out = nc.dram_tensor("out", [BPC, R, F], fp32, kind="ExternalOutput")

    with tile.TileContext(nc) as tc:
        with tc.tile_pool(name="metap", bufs=BPC) as mpool, \
             tc.tile_pool(name="useq", bufs=BPC) as upool, \
             tc.tile_pool(name="u23p", bufs=8) as u23pool, \
             tc.tile_pool(name="vload", bufs=10) as vpool, \
             tc.tile_pool(name="ohot", bufs=8) as opool, \
             tc.tile_pool(name="outreg", bufs=2) as rpool, \
             tc.tile_pool(name="ps1", bufs=4, space="PSUM") as ps1pool, \
             tc.tile_pool(name="ps23", bufs=4, space="PSUM") as ps23pool:

            vtiles = {}

            io_t = mpool.tile([P, P], fp32, tag="iota")
            nc.sync.dma_start(out=io_t[:], in_=iota[:])

            # prefetch all batches' metadata up front (small, keeps the
            # batch-transition critical path off the DMA queue)
            metas = []
            for b in range(BPC):
                m = mpool.tile([P, NSLOT, NMC], fp32, tag="m")
                nc.sync.dma_start(out=m[:], in_=meta[b])
                x1 = mpool.tile([P, NG, 8], i16, tag="x1")
                nc.sync.dma_start(out=x1[:], in_=idx1[b])
                x23 = mpool.tile([P, 8, 16], i16, tag="x23")
                nc.sync.dma_start(out=x23[:], in_=idx23[b])
                u1 = upool.tile([P, NSLOT], vdt, tag="u1")
                nc.vector.tensor_tensor(
                    out=u1[:], in0=m[:, :, MC_WA], in1=m[:, :, MC_OPA], op=MUL)
                u1f = upool.tile([P, NSLOT], fp32, tag="u1f")
                nc.vector.tensor_tensor(
                    out=u1f[:], in0=m[:, :, MC_WA], in1=m[:, :, MC_OPA], op=MUL)
                metas.append((m, x1, x23, u1, u1f))

            for b in range(BPC):
                m, x1, x23, u1, u1f = metas[b]
                outreg = rpool.tile([P, 32 * P], fp32)

                def vtile(t):
                    tg = b * TILES_PER_BATCH + t
                    g = tg // VB
                    if g not in vtiles:
                        vt = vpool.tile([P, VB, F], vdt, tag="v")
                        nc.sync.dma_start(out=vt[:], in_=vals[g])
                        vtiles[g] = vt
                    return vtiles[g][:, tg % VB, :]

                for g in range(NG):
                    lower = g < 8
                    ntiles = LOW_TPG if lower else UP_CAP
                    ps1 = ps1pool.tile([P, F], fp32, tag="ps1")
                    # group one-hot slabs: GPSIMD local scatter, except a
                    # share of upper groups built per-tile on the Vector
                    # engine to balance the two
                    o1s = opool.tile([P, 8 * P], vdt, tag="o1s")
                    if lower or g % 2 == 0:
                        nc.gpsimd.local_scatter(
                            out_ap=o1s[:, :ntiles * P],
                            data_ap=u1[:, g * 8:g * 8 + 8],
                            idxs_ap=x1[:, g, :],
                            channels=P, num_elems=ntiles * P, num_idxs=8)
                    else:
                        for tloc in range(ntiles):
                            s = g * 8 + tloc
                            nc.vector.tensor_scalar(
                                out=o1s[:, tloc * P:(tloc + 1) * P],
                                in0=io_t[:],
                                scalar1=m[:, s, MC_R1:MC_R1 + 1],
                                scalar2=u1f[:, s:s + 1],
                                op0=EQ, op1=MUL)
                    if lower:
                        u23g = u23pool.tile([P, 16], vdt, tag="u23g")
                        nc.vector.tensor_tensor(
                            out=u23g[:, 0:8],
                            in0=m[:, g * 8:g * 8 + 8, MC_WB],
                            in1=m[:, g * 8:g * 8 + 8, MC_OP2], op=MUL)
                        nc.vector.tensor_tensor(
                            out=u23g[:, 8:16],
                            in0=m[:, g * 8:g * 8 + 8, MC_WC],
                            in1=m[:, g * 8:g * 8 + 8, MC_OP2], op=MUL)
                        # cons one-hot ranges: per pair q, 4 ranges of 128
                        # cols: [T0->blkA, T1A->blkA, T1B->blkB, T2->blkB]
                        o23s = opool.tile([P, 8 * P], vdt, tag="o23s")
                        nc.gpsimd.local_scatter(
                            out_ap=o23s[:], data_ap=u23g[:],
                            idxs_ap=x23[:, g, :],
                            channels=P, num_elems=8 * P, num_idxs=16)
                    if lower:
                        i1 = 0
                        for q in range(2):
                            tau = 3 * q
                            vv = [vtile(_tile_of(g, tau + j)) for j in range(3)]
                            psA = ps23pool.tile([P, F], fp32, tag="ps23")
                            nc.tensor.matmul(
                                out=ps1[:], lhsT=o1s[:, tau * P:(tau + 1) * P],
                                rhs=vv[0], start=(i1 == 0), stop=False)
                            i1 += 1
                            nc.tensor.matmul(
                                out=psA[:], lhsT=o23s[:, (4 * q) * P:(4 * q + 1) * P],
                                rhs=vv[0], start=True, stop=False)
                            nc.tensor.matmul(
                                out=ps1[:], lhsT=o1s[:, (tau + 1) * P:(tau + 2) * P],
                                rhs=vv[1], start=False, stop=False)
                            i1 += 1
                            nc.tensor.matmul(
                                out=psA[:], lhsT=o23s[:, (4 * q + 1) * P:(4 * q + 2) * P],
                                rhs=vv[1], start=False, stop=True)
                            kA = 4 * g + 2 * q
                            nc.scalar.copy(
                                out=outreg[:, kA * P:(kA + 1) * P], in_=psA[:])
                            psB = ps23pool.tile([P, F], fp32, tag="ps23")
                            nc.tensor.matmul(
                                out=psB[:], lhsT=o23s[:, (4 * q + 2) * P:(4 * q + 3) * P],
                                rhs=vv[1], start=True, stop=False)
                            nc.tensor.matmul(
                                out=ps1[:], lhsT=o1s[:, (tau + 2) * P:(tau + 3) * P],
                                rhs=vv[2], start=False, stop=(i1 == LOW_TPG - 1))
                            i1 += 1
                            nc.tensor.matmul(
                                out=psB[:], lhsT=o23s[:, (4 * q + 3) * P:(4 * q + 4) * P],
                                rhs=vv[2], start=False, stop=True)
                            kB = kA + 1
                            nc.scalar.copy(
                                out=outreg[:, kB * P:(kB + 1) * P], in_=psB[:])
                    else:
                        for tloc in range(ntiles):
                            v = vtile(_tile_of(g, tloc))
                            nc.tensor.matmul(
                                out=ps1[:], lhsT=o1s[:, tloc * P:(tloc + 1) * P],
                                rhs=v, start=(tloc == 0),
                                stop=(tloc == ntiles - 1))
                    # car/cdr drain: bins [128g, +128) add onto cons copy
                    nc.vector.tensor_tensor(
                        out=outreg[:, g * P:(g + 1) * P],
                        in0=outreg[:, g * P:(g + 1) * P], in1=ps1[:], op=ADD)

                    # flush finished bin-blocks early to shorten the tail:
                    # after g7, blocks 0-7 (car/cdr done) and 16-31 (cons
                    # only) are final; blocks 8-15 finalize at their group.
                    def flush(k0, k1):
                        nc.sync.dma_start(
                            out=out[b, k0 * P:k1 * P, :]
                            .rearrange("(k p) f -> p k f", p=P),
                            in_=outreg[:, k0 * P:k1 * P]
                            .rearrange("p (k f) -> p k f", f=F))
                    if g == 7:
                        flush(0, 8)
                        flush(16, 32)
                    elif g == 11:
                        flush(8, 12)
                    elif g == 15:
                        flush(12, 16)

    nc.compile()
    return nc


def _pack_inputs(mem_values, arg_weights, root_filler, op_dist,
                 batch_idx, slot_idx, role_idx):
    """Host-side sharding/packing. Index selection and copies only."""
    mem_values = np.ascontiguousarray(mem_values, dtype=np.float32)
    arg_weights = np.asarray(arg_weights, dtype=np.float32)
    root_filler = np.asarray(root_filler, dtype=np.float32)
    op_dist = np.asarray(op_dist, dtype=np.float32)
    batch_idx = np.asarray(batch_idx, dtype=np.int64)
    slot_idx = np.asarray(slot_idx, dtype=np.int64)
    role_idx = np.asarray(role_idx, dtype=np.int64)

    # per-entry selected copies (pure gathers, no arithmetic)
    w = arg_weights[batch_idx, slot_idx]  # [N, 4] copies
    r = role_idx
    even = (r & 1) == 0
    wA = np.where(even, w[:, 0], np.where(r != 1, w[:, 1], 0.0)).astype(np.float32)
    opA = np.where(even, op_dist[batch_idx, 0],
                   op_dist[batch_idx, 1]).astype(np.float32)
    lo = r < H
    wB = np.where(lo, w[:, 2], 0.0).astype(np.float32)
    wC = np.where(lo, w[:, 3], 0.0).astype(np.float32)
    op2c = op_dist[batch_idx, 2].astype(np.float32)

    # block id within batch: lower cons blocks 0..31 (64 r each),
    # upper blocks 32..39 (256 r each)
    blk = np.where(lo, r >> 6, 32 + ((r - H) >> 8))

    vdt = np.dtype(CONFIG["val_dtype"])
    VB = CONFIG["vload_batch"]
    in_maps = []
    for c in range(NCORES):
        vals_s = np.zeros((NT * P, F), vdt)
        # entry-indexed (tile space) scratch, converted to slot space below
        r1_rel = np.full((NT, P), -1, np.int64)
        r23_rel = np.full((NT, P), -1, np.int64)
        wA_t = np.zeros((NT, P), np.float32)
        opA_t = np.zeros((NT, P), np.float32)
        wB_t = np.zeros((NT, P), np.float32)
        wC_t = np.zeros((NT, P), np.float32)
        op2_t = np.zeros((NT, P), np.float32)
        rho_t = np.full((NT, P), -1, np.int64)  # cons col-range per entry
        for bb in range(BPC):
            b = c * BPC + bb
            sel = np.nonzero(batch_idx == b)[0]
            gb = blk[sel]
            order = np.argsort(gb, kind="stable")
            sel = sel[order]
            gb = gb[order]
            counts = np.bincount(gb, minlength=40)
            counts_root = counts.copy()
            counts_root[0] += 1  # synthetic root entry joins block 0
            pair_sum = counts_root[:32].reshape(16, 2).sum(1)
            if (counts_root[:32] > BLK_CAP).any() or \
               (pair_sum > PAIR_CAP).any() or \
               (counts_root[32:] > UP_CAP * P).any():
                raise RuntimeError(
                    "static schedule capacity exceeded: "
                    f"lower={counts_root[:32].max()} pair={pair_sum.max()} "
                    f"upper={counts_root[32:].max()}")
            first = np.concatenate([[0], np.cumsum(counts)])[:-1]
            pos = np.arange(sel.size) - first[gb]

            def place(gbv, posv):
                """(block, pos-in-block) -> (tile-in-batch, partition,
                cons col-range rho or -1). Lower pairs straddle-packed:
                T0 pure-A, T2 pure-B, T1 = A overflow then B overflow."""
                low = gbv < 32
                gg = gbv >> 2
                qq = (gbv >> 1) & 1
                side = gbv & 1
                ov = posv >= P
                cA = counts_root[np.clip(gbv & ~1, 0, 39)]
                cAover = np.maximum(cA - P, 0)
                tau_lo = np.where(ov, 3 * qq + 1,
                                  np.where(side == 0, 3 * qq, 3 * qq + 2))
                part_lo = np.where(~ov, posv,
                                   np.where(side == 0, posv - P,
                                            cAover + posv - P))
                rho_lo = 4 * qq + np.where(
                    ov, np.where(side == 0, 1, 2),
                    np.where(side == 0, 0, 3))
                tile_lo = gg * LOW_TPG + tau_lo
                ug = gbv - 32
                tile_up = 8 * LOW_TPG + ug * UP_CAP + posv // P
                tile = np.where(low, tile_lo, tile_up)
                part = np.where(low, part_lo, posv % P)
                rho = np.where(low, rho_lo, -1)
                return tile, part, rho

            tile_a, part_a, rho_a = place(gb, pos)
            tix = bb * TILES_PER_BATCH + tile_a
            pix = part_a
            vals_s[tix * P + pix] = mem_values[sel]
            rr = role_idx[sel]
            r1_rel[tix, pix] = (rr >> 1) & 127
            r23_rel[tix, pix] = np.where(rr < H, rr & 63, -1)
            rho_t[tix, pix] = rho_a
            wA_t[tix, pix] = wA[sel]
            opA_t[tix, pix] = opA[sel]
            wB_t[tix, pix] = wB[sel]
            wC_t[tix, pix] = wC[sel]
            op2_t[tix, pix] = op2c[sel]
            # synthetic root entry -> bin 1 == 2*0+1 (block 0, odd cons)
            rt, rp, rrho = place(np.array([0]), np.array([counts[0]]))
            ti = bb * TILES_PER_BATCH + rt[0]
            pi = rp[0]
            vals_s[ti * P + pi] = root_filler[b]
            r1_rel[ti, pi] = -1
            r23_rel[ti, pi] = 0
            rho_t[ti, pi] = rrho[0]
            wC_t[ti, pi] = 1.0
            op2_t[ti, pi] = op_dist[b, 2]

        # tile space -> slot space
        meta_s = np.zeros((BPC, NSLOT, P, NMC), np.float32)
        idx1_s = np.full((BPC, NG, P, 8), -1, np.int16)
        idx23_s = np.full((BPC, 8, P, 16), -1, np.int16)
        for bb in range(BPC):
            for g in range(NG):
                ntl = LOW_TPG if g < 8 else UP_CAP
                for tloc in range(ntl):
                    t = bb * TILES_PER_BATCH + _tile_of(g, tloc)
                    s = _slot_of(g, tloc)
                    meta_s[bb, s, :, MC_WA] = wA_t[t]
                    meta_s[bb, s, :, MC_OPA] = opA_t[t]
                    meta_s[bb, s, :, MC_WB] = wB_t[t]
                    meta_s[bb, s, :, MC_WC] = wC_t[t]
                    meta_s[bb, s, :, MC_OP2] = op2_t[t]
                    meta_s[bb, s, :, MC_R1] = r1_rel[t]
                    meta_s[bb, s, :, MC_R23] = r23_rel[t]
                    v1 = r1_rel[t] >= 0
                    idx1_s[bb, g, :, tloc] = np.where(
                        v1, tloc * P + r1_rel[t], -1)
                    if g < 8:
                        v23 = r23_rel[t] >= 0
                        base = rho_t[t] * P + 2 * r23_rel[t]
                        idx23_s[bb, g, :, tloc] = np.where(v23, base, -1)
                        idx23_s[bb, g, :, 8 + tloc] = np.where(v23, base + 1, -1)

        in_maps.append({
            # [NT*P, F] -> [NT//VB, P, VB, F] load-grouped layout
            "vals": np.ascontiguousarray(
                vals_s.reshape(NT // VB, VB, P, F).transpose(0, 2, 1, 3)),
            # partition-major layouts so each partition's DMA is contiguous
            "meta": np.ascontiguousarray(meta_s.transpose(0, 2, 1, 3)),
            "idx1": np.ascontiguousarray(idx1_s.transpose(0, 2, 1, 3)),
            "idx23": np.ascontiguousarray(idx23_s.transpose(0, 2, 1, 3)),
            "iota": np.broadcast_to(
                np.arange(P, dtype=np.float32), (P, P)).copy(),
        })
    return in_maps


def kernel(**inputs):
    from concourse.bass_utils import run_bass_kernel_spmd

    in_maps = _pack_inputs(**inputs)
    if "nc" not in _PROG_CACHE:
        _PROG_CACHE["nc"] = _build_program()
    nc = _PROG_CACHE["nc"]
    res = run_bass_kernel_spmd(nc, in_maps, list(range(NCORES)))
    return np.concatenate([res.results[c]["out"] for c in range(NCORES)], axis=0)
